# revision 27
# baseline (speedup 1.0000x reference)
"""Trainium2 Bass kernel for a dense transformer encoder block (B=4, S=2048,
D=1024, H=16, MLP=4096).

Sharding: 8 cores = 4 batch elements x 2 query-halves, no collectives. Each
core's kv sequence is host-reordered so its 1024 query tokens come first
(attention is permutation-invariant over keys), so Q/residual tensors are
plain slices of the kv set. K/V are computed for the full 2048-token sequence
(~6% duplicated FLOPs vs. perfect sharding).

Per-core dataflow is feature-major ("T" = [feature, token]) so every matmul
has contraction dim 128 on partitions (sub-128-contraction matmuls fail to
load on this stack, all dtypes):
  LN1 (token-major, bn_stats) -> PE-transpose -> xnT            [phase 1]
  per head-group of 4 heads: Q/K/V projections from xnT         [phase 3]
    scores^T = KT_pair^T @ Qpad   (zero-padded rhs selects one head
                                   of the packed pair; K=128 kept)
    exp on ACT, scale=1/8 fused, both heads in one [128,1024] op -> f32r
    AV+den fused: lhsT = [V_head | 1 | 0] so psum rows 0:64 = V^T e and
      row 64 = sum(e); one augmented matmul per (ktile, head)
    reciprocal of row 64, partition-broadcast via DRAM round-trip DMA
      (stride-0 partition APs are DRAM-only), multiply -> RT; head B's
      rows shift 0:64 -> 64:128 via a small SBUF->SBUF DMA
  O-proj +bo, PE-transpose back, +residual -> x2 -> DRAM        [phase 4a]
  LN2 on x2 -> PE-transpose -> xn2T                             [phase 4b]
  MLP: h1 (+b1 and exact-erf Gelu fused on ACT), h2 (+b2),      [phase 5]
    PE-transpose back, +x2 residual -> out

Numerics: matmuls in float32r (TF32-class, ~1.5e-4 rel err, full PE rate at
free-dim >= 256; requires producers typed f32r), fp32 PSUM accumulation,
fp32 layernorm/softmax scalars. End-to-end rel err ~1.4e-4.

Weights are host-retiled to [tile, partition, kd, m] so each weight-tile DMA
is one contiguous block (4KB per-partition chunks). LN affine (g=1, b=0 for
this problem's inputs) is skipped at build time when the host detects
identity values; a full-affine variant is built otherwise.

Cost-model (TimelineSim) span: ~876 us/core; PE busy ~820 us (the binding
engine; attention runs at 50% array utilization, the price of the K=128
constraint with DH=64 heads and no working sub-128 row/col tiling).
"""

import os
import sys

sys.path.insert(0, "/opt/trn_rl_repo")

from contextlib import ExitStack

import numpy as np

import concourse.bass as bass
import concourse.tile as tile
from concourse import bacc, bass_utils, mybir
from concourse.masks import make_identity

F32 = mybir.dt.float32
F32R = mybir.dt.float32r
BF16 = mybir.dt.bfloat16
AF = mybir.ActivationFunctionType
ALU = mybir.AluOpType

B, S, D = 4, 2048, 1024
H, DH, MLP = 16, 64, 4096
P = 128
KD = D // P            # 8 partition tiles over D
FT = MLP // P          # 32 partition tiles over MLP dim
NQ = S // 2            # 1024 query tokens per core
ST = S // P            # 16 kv token tiles
QTT = NQ // P          # 8 q token tiles
QS = 512               # free-dim slice
NQS = NQ // QS         # 2
NKS = S // QS          # 4
NG = 4                 # head groups
EPS = 1e-6
DEBUG = bool(int(os.environ.get("KERNEL_DEBUG", "0")))
MLP_BF16 = bool(int(os.environ.get("KERNEL_MLP_BF16", "0")))

_CACHE = {}


def _build(ln_affine=True, mlp_bf16=True):
    nc = bacc.Bacc(None, target_bir_lowering=False, debug=False, num_devices=8)

    xkv = nc.dram_tensor("xkv", [S, D], F32, kind="ExternalInput").ap()
    # weights arrive host-tiled: [tile, p, kd, m] so each SBUF weight tile is
    # one contiguous DRAM block (4KB+ per-partition DMA chunks)
    Wq = nc.dram_tensor("Wq", [KD, P, KD, P], F32R, kind="ExternalInput").ap()
    Wk = nc.dram_tensor("Wk", [KD, P, KD, P], F32R, kind="ExternalInput").ap()
    Wv = nc.dram_tensor("Wv", [NG, P, KD, 256], F32R, kind="ExternalInput").ap()
    Wo = nc.dram_tensor("Wo", [KD, P, KD, P], F32R, kind="ExternalInput").ap()
    W1 = nc.dram_tensor("W1", [FT, P, KD, P], F32R, kind="ExternalInput").ap()
    W2 = nc.dram_tensor("W2", [KD, P, FT, P], BF16 if mlp_bf16 else F32R, kind="ExternalInput").ap()
    bq = nc.dram_tensor("bq", [D], F32, kind="ExternalInput").ap()
    bk = nc.dram_tensor("bk", [D], F32, kind="ExternalInput").ap()
    bv = nc.dram_tensor("bv", [D], F32, kind="ExternalInput").ap()
    bo = nc.dram_tensor("bo", [D], F32, kind="ExternalInput").ap()
    b1 = nc.dram_tensor("b1", [MLP], F32, kind="ExternalInput").ap()
    b2 = nc.dram_tensor("b2", [D], F32, kind="ExternalInput").ap()
    g1 = nc.dram_tensor("g1", [D], F32, kind="ExternalInput").ap()
    be1 = nc.dram_tensor("be1", [D], F32, kind="ExternalInput").ap()
    g2 = nc.dram_tensor("g2", [D], F32, kind="ExternalInput").ap()
    be2 = nc.dram_tensor("be2", [D], F32, kind="ExternalInput").ap()
    out = nc.dram_tensor("out", [NQ, D], F32, kind="ExternalOutput").ap()

    dbg = {}
    if DEBUG:
        dbg["xnkvT"] = nc.dram_tensor("d_xnkvT", [P, KD, S], F32R, kind="ExternalOutput").ap()
        dbg["qt0"] = nc.dram_tensor("d_qt0", [P, 2, NQ], F32R, kind="ExternalOutput").ap()
        dbg["kt0"] = nc.dram_tensor("d_kt0", [P, 2, S], F32R, kind="ExternalOutput").ap()
        dbg["v0"] = nc.dram_tensor("d_v0", [P, ST, 2, 2, P], F32R, kind="ExternalOutput").ap()
        dbg["rt"] = nc.dram_tensor("d_rt", [P, KD, NQ], F32R, kind="ExternalOutput").ap()
        dbg["e0"] = nc.dram_tensor("d_e0", [P, QS], F32R, kind="ExternalOutput").ap()
        dbg["s0"] = nc.dram_tensor("d_s0", [P, QS], F32, kind="ExternalOutput").ap()
        dbg["av0"] = nc.dram_tensor("d_av0", [65, 2, QS], F32, kind="ExternalOutput").ap()
        dbg["x2"] = nc.dram_tensor("d_x2", [P, QTT, D], F32, kind="ExternalOutput").ap()

    def bcast_ap(vec):
        # [D] dram vector -> [128, D] partition-replicated DMA source
        return bass.AP(tensor=vec.tensor, offset=vec.offset, ap=[[0, P]] + list(vec.ap))



    with tile.TileContext(nc) as tc:
        es = ExitStack()
        params = es.enter_context(tc.tile_pool(name="params", bufs=1))
        dramp = es.enter_context(tc.tile_pool(name="dram", bufs=1, space="DRAM"))
        x2d = dramp.tile([P, QTT, D], F32)

        ident_f = params.tile([P, P], F32)
        make_identity(nc, ident_f)
        ident = params.tile([P, P], F32R)
        nc.vector.tensor_copy(ident[:], ident_f[:])
        ones_f = params.tile([P, 1], F32)
        nc.vector.memset(ones_f[:, 0:1], 1.0)

        def pvec(v, n, nm):  # [n*128] -> [128, n] (dim o*128+p -> [p, o])
            t = params.tile([P, n], F32, name=nm)
            nc.sync.dma_start(t[:], v.rearrange("(o p) -> p o", p=P))
            return t

        bq_t = pvec(bq, KD, "bq_t")
        bk_t = pvec(bk, KD, "bk_t")
        bo_t = pvec(bo, KD, "bo_t")
        b2_t = pvec(b2, KD, "b2_t")
        b1_t = pvec(b1, FT, "b1_t")
        bv_rep = params.tile([P, D], F32)
        nc.gpsimd.dma_start(bv_rep[:], bcast_ap(bv))

        rt_es = ExitStack()
        rtp = rt_es.enter_context(tc.tile_pool(name="rt", bufs=1))
        RT = rtp.tile([P, KD, NQ], F32R)

        xn_es = ExitStack()
        xnp = xn_es.enter_context(tc.tile_pool(name="xn", bufs=1))
        xn_kvT = xnp.tile([P, KD, S], F32R)

        # ---- Phase 1: LN1 + transpose to feature-major ----
        with tc.tile_pool(name="p1tmp", bufs=3) as p1t, \
             tc.tile_pool(name="p1s", bufs=4) as p1s, \
             tc.tile_pool(name="ln1", bufs=1) as ln1p, \
             tc.tile_pool(name="p1ps", bufs=4, space="PSUM") as ps1:
            g1_rep = ln1p.tile([P, D], F32)
            nc.gpsimd.dma_start(g1_rep[:], bcast_ap(g1))
            be1_rep = ln1p.tile([P, D], F32)
            nc.gpsimd.dma_start(be1_rep[:], bcast_ap(be1))
            eps_t = ln1p.tile([P, 1], F32)
            nc.vector.memset(eps_t[:], EPS)

            for t in range(ST):
                x_t = p1t.tile([P, D], F32, tag="x_t")
                nc.sync.dma_start(x_t[:], xkv[t * P:(t + 1) * P, :])
                stats = p1s.tile([P, 2, 6], F32, tag="stats")
                xv = x_t[:].rearrange("p (s f) -> p s f", s=2)
                for s in range(2):
                    nc.vector.bn_stats(stats[:, s, :], xv[:, s, :])
                mv = p1s.tile([P, 2], F32, tag="mv")
                nc.vector.bn_aggr(mv[:], stats[:])
                std = p1s.tile([P, 1], F32, tag="std")
                nc.scalar.activation(std[:], mv[:, 1:2], AF.Sqrt, bias=eps_t[:])
                nc.vector.reciprocal(std[:], std[:])
                xn_t = p1t.tile([P, D], F32R, tag="xn_t")
                nc.vector.tensor_scalar(
                    xn_t[:], x_t[:], scalar1=mv[:, 0:1], scalar2=std[:],
                    op0=ALU.subtract, op1=ALU.mult)
                if ln_affine:
                    nc.vector.tensor_tensor(xn_t[:], xn_t[:], g1_rep[:], ALU.mult)
                    nc.vector.tensor_tensor(xn_t[:], xn_t[:], be1_rep[:], ALU.add)
                for j in range(KD):
                    pst = ps1.tile([P, P], F32, tag="tp")
                    nc.tensor.transpose(pst[:].bitcast(F32R), xn_t[:, j * P:(j + 1) * P], ident[:])
                    nc.vector.tensor_copy(xn_kvT[:, j, t * P:(t + 1) * P], pst[:])

        if DEBUG:
            nc.sync.dma_start(dbg["xnkvT"], xn_kvT[:])

        # ---- Phase 3: per-group QKV projection + attention ----
        with tc.tile_pool(name="kv", bufs=1) as kvp, \
             tc.tile_pool(name="wst", bufs=2) as wsp, \
             tc.tile_pool(name="expp", bufs=2) as expp, \
             tc.tile_pool(name="qpad", bufs=1) as qpp, \
             tc.tile_pool(name="rcbc", bufs=1) as rcp, \
             tc.tile_pool(name="aps", bufs=1, space="PSUM") as aps:

            zsc = qpp.tile([P, QS], F32)
            nc.vector.memset(zsc[:], 0.0)
            qpadA = [qpp.tile([P, QS], F32R, name=f"qpadA{i}") for i in range(1)]
            qpadB = [qpp.tile([P, QS], F32R, name=f"qpadB{i}") for i in range(1)]
            for i in range(1):
                nc.vector.tensor_copy(qpadA[i][:], zsc[:])
                nc.vector.tensor_copy(qpadB[i][:], zsc[:])

            QT_g = kvp.tile([P, 2, NQ], F32R)
            KT_g = kvp.tile([P, 2, S], F32R)
            # per (toktile, pair, head j): [V_head(64) | 1 | 0(63)]
            V_gp = kvp.tile([P, ST, 2, 2, P], F32R)
            for t in range(ST):
                nc.vector.tensor_copy(
                    V_gp[:, t], zsc[:].rearrange("p (a b m) -> p a b m", a=2, b=2))
            one_r = qpp.tile([P, 1], F32R)
            nc.vector.tensor_copy(one_r[:], ones_f[:, 0:1])
            for t in range(ST):
                for pi in range(2):
                    for j in range(2):
                        nc.vector.tensor_copy(V_gp[:, t, pi, j, 64:65], one_r[:])
            it_count = 0

            for g in range(NG):
                for pl in range(2):   # head pairs 2g, 2g+1
                    pr = 2 * g + pl
                    wq_t = wsp.tile([P, KD, P], F32R, tag="wq_t")
                    nc.sync.dma_start(wq_t[:], Wq[pr])
                    for q in range(NQS):
                        ps = aps.tile([P, QS], F32, tag="pp", bufs=2)
                        for kd in range(KD):
                            nc.tensor.matmul(
                                ps[:], wq_t[:, kd, :], xn_kvT[:, kd, q * QS:(q + 1) * QS],
                                start=(kd == 0), stop=(kd == KD - 1))
                        nc.vector.tensor_scalar_add(
                            QT_g[:, pl, q * QS:(q + 1) * QS], ps[:], bq_t[:, pr:pr + 1])
                    wk_t = wsp.tile([P, KD, P], F32R, tag="wk_t")
                    nc.sync.dma_start(wk_t[:], Wk[pr])
                    for q in range(NKS):
                        ps = aps.tile([P, QS], F32, tag="pp", bufs=2)
                        for kd in range(KD):
                            nc.tensor.matmul(
                                ps[:], wk_t[:, kd, :], xn_kvT[:, kd, q * QS:(q + 1) * QS],
                                start=(kd == 0), stop=(kd == KD - 1))
                        nc.vector.tensor_scalar_add(
                            KT_g[:, pl, q * QS:(q + 1) * QS], ps[:], bk_t[:, pr:pr + 1])
                wv_t = wsp.tile([P, KD, 256], F32R, tag="wv_t", bufs=1)
                nc.sync.dma_start(wv_t[:], Wv[g])
                for t in range(ST):
                    ps = aps.tile([P, QS], F32, tag="pp", bufs=2)
                    for kd in range(KD):
                        nc.tensor.matmul(
                            ps[:, 0:256], xn_kvT[:, kd, t * P:(t + 1) * P], wv_t[:, kd, :],
                            start=(kd == 0), stop=(kd == KD - 1))
                    for pi in range(2):
                        nc.vector.tensor_tensor(
                            V_gp[:, t, pi, :, 0:64],
                            ps[:, pi * 128:(pi + 1) * 128].rearrange("p (j m) -> p j m", j=2),
                            bv_rep[:, g * 256 + pi * 128:g * 256 + (pi + 1) * 128].rearrange(
                                "p (j m) -> p j m", j=2), ALU.add)

                if DEBUG and g == 0:
                    nc.sync.dma_start(dbg["kt0"], KT_g[:])
                    nc.sync.dma_start(dbg["v0"], V_gp[:])
                    nc.sync.dma_start(dbg["qt0"], QT_g[:])

                for q in range(NQS):
                    for pl in range(2):
                        pr = 2 * g + pl
                        i = it_count % 1
                        it_count += 1
                        qsl = slice(q * QS, (q + 1) * QS)
                        nc.vector.tensor_copy(qpadA[i][0:64, :], QT_g[0:64, pl, qsl])
                        nc.vector.tensor_copy(qpadB[i][64:128, :], QT_g[64:128, pl, qsl])
                        av1 = aps.tile([P, QS], F32, tag="av1")
                        av2 = aps.tile([P, QS], F32, tag="av2")
                        for kt in range(ST):
                            ktsl = slice(kt * P, (kt + 1) * P)
                            sAB = aps.tile([P, 2, QS], F32, tag="sAB", bufs=2)
                            nc.tensor.matmul(sAB[:, 0, :], KT_g[:, pl, ktsl], qpadA[i][:],
                                             start=True, stop=True)
                            nc.tensor.matmul(sAB[:, 1, :], KT_g[:, pl, ktsl], qpadB[i][:],
                                             start=True, stop=True)
                            eAB = expp.tile([P, 2, QS], F32R, tag="eAB")
                            nc.scalar.activation(eAB[:], sAB[:], AF.Exp, scale=0.125)
                            eA = eAB[:, 0, :]
                            eB = eAB[:, 1, :]
                            if DEBUG and g == 0 and q == 0 and pl == 0 and kt == 0:
                                nc.sync.dma_start(dbg["e0"], eA)
                                s0c = rcp.tile([P, QS], F32, tag="s0c")
                                nc.vector.tensor_copy(s0c[:], sAB[:, 0, :])
                                nc.sync.dma_start(dbg["s0"], s0c[:])
                            st, sp = (kt == 0), (kt == ST - 1)
                            nc.tensor.matmul(av1[:], V_gp[:, kt, pl, 0, :], eA,
                                             start=st, stop=sp, skip_group_check=True)
                            nc.tensor.matmul(av2[:], V_gp[:, kt, pl, 1, :], eB,
                                             start=st, stop=sp, skip_group_check=True)
                        # free the av psums fast: copy to SBUF, divide from there
                        avc = rcp.tile([65, 2, QS], F32, tag="avc")
                        nc.vector.tensor_copy(avc[0:65, 0, :], av1[0:65, :])
                        nc.vector.tensor_copy(avc[0:65, 1, :], av2[0:65, :])
                        nc.vector.reciprocal(avc[64:65, 0, :], avc[64:65, 0, :])
                        nc.vector.reciprocal(avc[64:65, 1, :], avc[64:65, 1, :])
                        rcd = dramp.tile([2, QS], F32, tag="rcd", bufs=2)
                        nc.sync.dma_start(rcd[0:1, :], avc[64:65, 0, :])
                        nc.sync.dma_start(rcd[1:2, :], avc[64:65, 1, :])
                        bcA = rcp.tile([64, QS], F32, tag="bcA")
                        bcB = rcp.tile([64, QS], F32, tag="bcB")

                        def _b64(row_ap):
                            return bass.AP(tensor=row_ap.tensor, offset=row_ap.offset,
                                           ap=[[0, 64]] + list(row_ap.ap)[1:])

                        nc.sync.dma_start(bcA[:], _b64(rcd[0:1, :]))
                        nc.sync.dma_start(bcB[:], _b64(rcd[1:2, :]))
                        if DEBUG and g == 0 and q == 0 and pl == 0:
                            nc.sync.dma_start(dbg["av0"], avc[:])
                        nc.vector.tensor_tensor(RT[0:64, pr, qsl], avc[0:64, 0, :], bcA[:], ALU.mult)
                        stB = rcp.tile([64, QS], F32R, tag="stB")
                        nc.vector.tensor_tensor(stB[:], avc[0:64, 1, :], bcB[:], ALU.mult)
                        nc.sync.dma_start(RT[64:128, pr, qsl], stB[:])

        xn_es.close()

        if DEBUG:
            nc.sync.dma_start(dbg["rt"], RT[:])

        # ---- Phase 4a: O-projection + residual -> x2 (DRAM) ----
        with tc.tile_pool(name="p4tmp", bufs=2) as p4t, \
             tc.tile_pool(name="p4ps", bufs=2, space="PSUM") as ps4, \
             tc.tile_pool(name="p4tps", bufs=4, space="PSUM") as ps4t:
            for q in range(NQS):
                attnT = p4t.tile([P, KD, QS], F32R, tag="attnT")
                for mt in range(KD):
                    wo_t = p4t.tile([P, KD, P], F32R, tag="wo_t")
                    nc.sync.dma_start(wo_t[:], Wo[mt])
                    ps = ps4.tile([P, QS], F32, tag="pp")
                    for kd in range(KD):
                        nc.tensor.matmul(
                            ps[:], wo_t[:, kd, :], RT[:, kd, q * QS:(q + 1) * QS],
                            start=(kd == 0), stop=(kd == KD - 1))
                    nc.vector.tensor_scalar_add(
                        attnT[:, mt, :], ps[:], bo_t[:, mt:mt + 1])
                for j in range(QS // P):
                    tt = q * (QS // P) + j
                    xr_t = p4t.tile([P, D], F32, tag="xr_t")
                    nc.sync.dma_start(xr_t[:], xkv[tt * P:(tt + 1) * P, :])
                    x2_t = p4t.tile([P, D], F32, tag="x2_t")
                    for mt in range(KD):
                        pst = ps4t.tile([P, P], F32, tag="tp")
                        nc.tensor.transpose(pst[:].bitcast(F32R),
                                            attnT[:, mt, j * P:(j + 1) * P], ident[:])
                        nc.vector.tensor_tensor(
                            x2_t[:, mt * P:(mt + 1) * P], pst[:],
                            xr_t[:, mt * P:(mt + 1) * P], ALU.add)
                    nc.sync.dma_start(x2d[:, tt, :], x2_t[:])
                    if DEBUG:
                        nc.sync.dma_start(dbg["x2"][:, tt, :], x2_t[:])
        rt_es.close()

        # ---- Phase 4b: LN2 -> xn2T ----
        xn2_es = ExitStack()
        xn2p = xn2_es.enter_context(tc.tile_pool(name="xn2", bufs=1))
        xn2T = xn2p.tile([P, KD, NQ], F32R)
        with tc.tile_pool(name="p4btmp", bufs=3) as p4bt, \
             tc.tile_pool(name="p4bs", bufs=4) as p4bs, \
             tc.tile_pool(name="ln2", bufs=1) as ln2p, \
             tc.tile_pool(name="p4bps", bufs=4, space="PSUM") as ps4b:
            g2_rep = ln2p.tile([P, D], F32)
            nc.gpsimd.dma_start(g2_rep[:], bcast_ap(g2))
            be2_rep = ln2p.tile([P, D], F32)
            nc.gpsimd.dma_start(be2_rep[:], bcast_ap(be2))
            eps2_t = ln2p.tile([P, 1], F32)
            nc.vector.memset(eps2_t[:], EPS)

            for tt in range(QTT):
                x2_t = p4bt.tile([P, D], F32, tag="x2_t")
                nc.sync.dma_start(x2_t[:], x2d[:, tt, :])
                stats = p4bs.tile([P, 2, 6], F32, tag="stats2")
                xv = x2_t[:].rearrange("p (s f) -> p s f", s=2)
                for s in range(2):
                    nc.vector.bn_stats(stats[:, s, :], xv[:, s, :])
                mv = p4bs.tile([P, 2], F32, tag="mv2")
                nc.vector.bn_aggr(mv[:], stats[:])
                std = p4bs.tile([P, 1], F32, tag="std2")
                nc.scalar.activation(std[:], mv[:, 1:2], AF.Sqrt, bias=eps2_t[:])
                nc.vector.reciprocal(std[:], std[:])
                xn2_t = p4bt.tile([P, D], F32R, tag="xn2_t")
                nc.vector.tensor_scalar(
                    xn2_t[:], x2_t[:], scalar1=mv[:, 0:1], scalar2=std[:],
                    op0=ALU.subtract, op1=ALU.mult)
                if ln_affine:
                    nc.vector.tensor_tensor(xn2_t[:], xn2_t[:], g2_rep[:], ALU.mult)
                    nc.vector.tensor_tensor(xn2_t[:], xn2_t[:], be2_rep[:], ALU.add)
                for j in range(KD):
                    pst = ps4b.tile([P, P], F32, tag="tp")
                    nc.tensor.transpose(pst[:].bitcast(F32R), xn2_t[:, j * P:(j + 1) * P], ident[:])
                    nc.vector.tensor_copy(xn2T[:, j, tt * P:(tt + 1) * P], pst[:])

        # ---- Phase 5: MLP (h1 in bf16, single full-width token pass) ----
        with tc.tile_pool(name="p5tmp", bufs=3) as p5t, \
             tc.tile_pool(name="h1", bufs=1) as h1p, \
             tc.tile_pool(name="w2st", bufs=2) as w2p, \
             tc.tile_pool(name="p5ps", bufs=2, space="PSUM") as ps5, \
             tc.tile_pool(name="p5tps", bufs=4, space="PSUM") as ps5t:
            mdt = BF16 if mlp_bf16 else F32R
            n_hslice = 1 if mlp_bf16 else NQS
            HW_ = NQ // n_hslice
            out_acc = h1p.tile([P, QTT, D], F32)
            for hs in range(n_hslice):
                h1T = h1p.tile([P, FT, HW_], mdt, tag="h1T")
                for ft in range(FT):
                    w1_t = p5t.tile([P, KD, P], F32R, tag="w1_t")
                    nc.sync.dma_start(w1_t[:], W1[ft])
                    for sl in range(HW_ // QS):
                        ssl = slice(hs * HW_ + sl * QS, hs * HW_ + (sl + 1) * QS)
                        ps = ps5.tile([P, QS], F32, tag="pp")
                        for kd in range(KD):
                            nc.tensor.matmul(
                                ps[:], w1_t[:, kd, :], xn2T[:, kd, ssl],
                                start=(kd == 0), stop=(kd == KD - 1))
                        nc.scalar.activation(h1T[:, ft, sl * QS:(sl + 1) * QS], ps[:],
                                             AF.Gelu, bias=b1_t[:, ft:ft + 1])
                for mt in range(KD):
                    w2_t = w2p.tile([P, FT, P], mdt, tag="w2_t")
                    nc.sync.dma_start(w2_t[:], W2[mt])
                    for sl in range(HW_ // QS):
                        ssl_loc = slice(sl * QS, (sl + 1) * QS)
                        ps = ps5.tile([P, QS], F32, tag="pp")
                        for ft in range(FT):
                            nc.tensor.matmul(
                                ps[:], w2_t[:, ft, :], h1T[:, ft, ssl_loc],
                                start=(ft == 0), stop=(ft == FT - 1))
                        outT = p5t.tile([P, QS], F32R, tag="outT", bufs=2)
                        nc.vector.tensor_scalar_add(outT[:], ps[:], b2_t[:, mt:mt + 1])
                        for j in range(QS // P):
                            tt = hs * (HW_ // P) + sl * (QS // P) + j
                            pst = ps5t.tile([P, P], F32, tag="tp")
                            nc.tensor.transpose(pst[:].bitcast(F32R),
                                                outT[:, j * P:(j + 1) * P], ident[:])
                            nc.vector.tensor_copy(out_acc[:, tt, mt * P:(mt + 1) * P], pst[:])
            for tt in range(QTT):
                x2_t = p5t.tile([P, D], F32, tag="x2r_t")
                nc.sync.dma_start(x2_t[:], x2d[:, tt, :])
                ob = p5t.tile([P, D], F32, tag="ob")
                nc.vector.tensor_tensor(ob[:], out_acc[:, tt, :], x2_t[:], ALU.add)
                nc.sync.dma_start(out[tt * P:(tt + 1) * P, :], ob[:])

        xn2_es.close()
        es.close()

    nc.compile()
    return nc


def kernel(**inputs):
    inputs = {k: np.ascontiguousarray(np.asarray(v), dtype=np.float32)
              for k, v in inputs.items()}
    ln_affine = not (
        np.all(inputs["ln1_g"] == 1.0) and np.all(inputs["ln1_b"] == 0.0)
        and np.all(inputs["ln2_g"] == 1.0) and np.all(inputs["ln2_b"] == 0.0))
    key = ("nc", ln_affine, MLP_BF16)
    if key not in _CACHE:
        _CACHE[key] = _build(ln_affine=ln_affine, mlp_bf16=MLP_BF16)
    nc = _CACHE[key]

    x = inputs["x"]
    def tile_w(W, n_out, m):
        # [Din, Dout] -> [Dout/m, 128, Din/128, m]
        Din, Dout = W.shape
        return np.ascontiguousarray(
            W.reshape(Din // P, P, n_out, m).transpose(2, 1, 0, 3))

    shared = {
        "Wq": tile_w(inputs["Wq"], KD, P), "Wk": tile_w(inputs["Wk"], KD, P),
        "Wv": tile_w(inputs["Wv"], NG, 256), "Wo": tile_w(inputs["Wo"], KD, P),
        "W1": tile_w(inputs["W1"], FT, P),
        "W2": (tile_w(inputs["W2"], KD, P).astype(__import__("ml_dtypes").bfloat16)
               if MLP_BF16 else tile_w(inputs["W2"], KD, P)),
        "bq": inputs["bq"], "bk": inputs["bk"], "bv": inputs["bv"], "bo": inputs["bo"],
        "b1": inputs["b1"], "b2": inputs["b2"],
        "g1": inputs["ln1_g"], "be1": inputs["ln1_b"],
        "g2": inputs["ln2_g"], "be2": inputs["ln2_b"],
    }
    in_maps = []
    for c in range(8):
        b, half = c // 2, c % 2
        m = dict(shared)
        # query half first; attention is permutation-invariant over kv order
        m["xkv"] = np.ascontiguousarray(
            np.concatenate([x[b, half * NQ:(half + 1) * NQ, :],
                            x[b, (1 - half) * NQ:(2 - half) * NQ, :]], axis=0))
        in_maps.append(m)

    trace = bool(int(os.environ.get("KERNEL_TRACE", "0")))
    kw = {}
    if trace:
        kw = dict(trace=True, tmpdir=os.environ.get("KERNEL_TRACE_DIR") or None)
    res = bass_utils.run_bass_kernel_spmd(nc, in_maps, core_ids=list(range(8)), **kw)
    _CACHE["last_results"] = res
    _CACHE["nc"] = nc
    _CACHE["last_in_maps"] = in_maps

    outa = np.empty((B, S, D), dtype=np.float32)
    for c in range(8):
        b, half = c // 2, c % 2
        outa[b, half * NQ:(half + 1) * NQ, :] = res.results[c]["out"]
    return outa


# revision 28
# speedup vs baseline: 1.0104x; 1.0104x over previous
"""Trainium2 Bass kernel for a dense transformer encoder block (B=4, S=2048,
D=1024, H=16, MLP=4096).

Sharding: 8 cores = 4 batch elements x 2 query-halves, no collectives. Each
core's kv sequence is host-reordered so its 1024 query tokens come first
(attention is permutation-invariant over keys), so Q/residual tensors are
plain slices of the kv set. K/V are computed for the full 2048-token sequence
(~6% duplicated FLOPs vs. perfect sharding).

Per-core dataflow is feature-major ("T" = [feature, token]) so every matmul
has contraction dim 128 on partitions (sub-128-contraction matmuls fail to
load on this stack, all dtypes):
  LN1 (token-major, bn_stats) -> PE-transpose -> xnT            [phase 1]
  per head-group of 4 heads: Q/K/V projections from xnT         [phase 3]
    scores^T = KT_pair^T @ Qpad   (zero-padded rhs selects one head
                                   of the packed pair; K=128 kept)
    exp on ACT, scale=1/8 fused, both heads in one [128,1024] op -> f32r
    AV+den fused: lhsT = [V_head | 1 | 0] so psum rows 0:64 = V^T e and
      row 64 = sum(e); one augmented matmul per (ktile, head)
    reciprocal of row 64, partition-broadcast via DRAM round-trip DMA
      (stride-0 partition APs are DRAM-only), multiply -> RT; head B's
      rows shift 0:64 -> 64:128 via a small SBUF->SBUF DMA
  O-proj +bo, PE-transpose back, +residual -> x2 -> DRAM        [phase 4a]
  LN2 on x2 -> PE-transpose -> xn2T                             [phase 4b]
  MLP: h1 (+b1 and exact-erf Gelu fused on ACT), h2 (+b2),      [phase 5]
    PE-transpose back, +x2 residual -> out

Numerics: matmuls in float32r (TF32-class, ~1.5e-4 rel err, full PE rate at
free-dim >= 256; requires producers typed f32r), fp32 PSUM accumulation,
fp32 layernorm/softmax scalars. End-to-end rel err ~1.4e-4.

Weights are host-retiled to [tile, partition, kd, m] so each weight-tile DMA
is one contiguous block (4KB per-partition chunks). LN affine (g=1, b=0 for
this problem's inputs) is skipped at build time when the host detects
identity values; a full-affine variant is built otherwise.

Cost-model (TimelineSim) span: ~876 us/core; PE busy ~820 us (the binding
engine; attention runs at 50% array utilization, the price of the K=128
constraint with DH=64 heads and no working sub-128 row/col tiling).
"""

import os
import sys

sys.path.insert(0, "/opt/trn_rl_repo")

from contextlib import ExitStack

import numpy as np

import concourse.bass as bass
import concourse.tile as tile
from concourse import bacc, bass_utils, mybir
from concourse.masks import make_identity

F32 = mybir.dt.float32
F32R = mybir.dt.float32r
BF16 = mybir.dt.bfloat16
AF = mybir.ActivationFunctionType
ALU = mybir.AluOpType

B, S, D = 4, 2048, 1024
H, DH, MLP = 16, 64, 4096
P = 128
KD = D // P            # 8 partition tiles over D
FT = MLP // P          # 32 partition tiles over MLP dim
NQ = S // 2            # 1024 query tokens per core
ST = S // P            # 16 kv token tiles
QTT = NQ // P          # 8 q token tiles
QS = 512               # free-dim slice
NQS = NQ // QS         # 2
NKS = S // QS          # 4
NG = 4                 # head groups
EPS = 1e-6
DEBUG = bool(int(os.environ.get("KERNEL_DEBUG", "0")))
MLP_BF16 = bool(int(os.environ.get("KERNEL_MLP_BF16", "0")))

_CACHE = {}


def _build(ln_affine=True, mlp_bf16=True):
    nc = bacc.Bacc(None, target_bir_lowering=False, debug=False, num_devices=8)

    xkv = nc.dram_tensor("xkv", [S, D], F32, kind="ExternalInput").ap()
    # weights arrive host-tiled: [tile, p, kd, m] so each SBUF weight tile is
    # one contiguous DRAM block (4KB+ per-partition DMA chunks)
    Wq = nc.dram_tensor("Wq", [KD, P, KD, P], F32R, kind="ExternalInput").ap()
    Wk = nc.dram_tensor("Wk", [KD, P, KD, P], F32R, kind="ExternalInput").ap()
    Wv = nc.dram_tensor("Wv", [NG, P, KD, 256], F32R, kind="ExternalInput").ap()
    Wo = nc.dram_tensor("Wo", [KD, P, KD, P], F32R, kind="ExternalInput").ap()
    W1 = nc.dram_tensor("W1", [FT, P, KD, P], F32R, kind="ExternalInput").ap()
    W2 = nc.dram_tensor("W2", [KD, P, FT, P], BF16 if mlp_bf16 else F32R, kind="ExternalInput").ap()
    bq = nc.dram_tensor("bq", [D], F32, kind="ExternalInput").ap()
    bk = nc.dram_tensor("bk", [D], F32, kind="ExternalInput").ap()
    bv = nc.dram_tensor("bv", [D], F32, kind="ExternalInput").ap()
    bo = nc.dram_tensor("bo", [D], F32, kind="ExternalInput").ap()
    b1 = nc.dram_tensor("b1", [MLP], F32, kind="ExternalInput").ap()
    b2 = nc.dram_tensor("b2", [D], F32, kind="ExternalInput").ap()
    g1 = nc.dram_tensor("g1", [D], F32, kind="ExternalInput").ap()
    be1 = nc.dram_tensor("be1", [D], F32, kind="ExternalInput").ap()
    g2 = nc.dram_tensor("g2", [D], F32, kind="ExternalInput").ap()
    be2 = nc.dram_tensor("be2", [D], F32, kind="ExternalInput").ap()
    out = nc.dram_tensor("out", [NQ, D], F32, kind="ExternalOutput").ap()

    dbg = {}
    if DEBUG:
        dbg["xnkvT"] = nc.dram_tensor("d_xnkvT", [P, KD, S], F32R, kind="ExternalOutput").ap()
        dbg["qt0"] = nc.dram_tensor("d_qt0", [P, 2, NQ], F32R, kind="ExternalOutput").ap()
        dbg["kt0"] = nc.dram_tensor("d_kt0", [P, 2, S], F32R, kind="ExternalOutput").ap()
        dbg["v0"] = nc.dram_tensor("d_v0", [P, ST, 2, 2, P], F32R, kind="ExternalOutput").ap()
        dbg["rt"] = nc.dram_tensor("d_rt", [P, KD, NQ], F32R, kind="ExternalOutput").ap()
        dbg["e0"] = nc.dram_tensor("d_e0", [P, QS], F32R, kind="ExternalOutput").ap()
        dbg["s0"] = nc.dram_tensor("d_s0", [P, QS], F32, kind="ExternalOutput").ap()
        dbg["av0"] = nc.dram_tensor("d_av0", [65, 2, QS], F32, kind="ExternalOutput").ap()
        dbg["x2"] = nc.dram_tensor("d_x2", [P, QTT, D], F32, kind="ExternalOutput").ap()

    def bcast_ap(vec):
        # [D] dram vector -> [128, D] partition-replicated DMA source
        return bass.AP(tensor=vec.tensor, offset=vec.offset, ap=[[0, P]] + list(vec.ap))



    with tile.TileContext(nc) as tc:
        es = ExitStack()
        params = es.enter_context(tc.tile_pool(name="params", bufs=1))
        dramp = es.enter_context(tc.tile_pool(name="dram", bufs=1, space="DRAM"))
        x2d = dramp.tile([P, QTT, D], F32)

        ident_f = params.tile([P, P], F32)
        make_identity(nc, ident_f)
        ident = params.tile([P, P], F32R)
        nc.vector.tensor_copy(ident[:], ident_f[:])
        ones_f = params.tile([P, 1], F32)
        nc.vector.memset(ones_f[:, 0:1], 1.0)

        def pvec(v, n, nm):  # [n*128] -> [128, n] (dim o*128+p -> [p, o])
            t = params.tile([P, n], F32, name=nm)
            nc.sync.dma_start(t[:], v.rearrange("(o p) -> p o", p=P))
            return t

        bq_t = pvec(bq, KD, "bq_t")
        bk_t = pvec(bk, KD, "bk_t")
        bo_t = pvec(bo, KD, "bo_t")
        b2_t = pvec(b2, KD, "b2_t")
        b1_t = pvec(b1, FT, "b1_t")
        bv_rep = params.tile([P, D], F32)
        nc.gpsimd.dma_start(bv_rep[:], bcast_ap(bv))

        rt_es = ExitStack()
        rtp = rt_es.enter_context(tc.tile_pool(name="rt", bufs=1))
        RT = rtp.tile([P, KD, NQ], F32R)

        xn_es = ExitStack()
        xnp = xn_es.enter_context(tc.tile_pool(name="xn", bufs=1))
        xn_kvT = xnp.tile([P, KD, S], F32R)

        # ---- Phase 1: LN1 + transpose to feature-major ----
        with tc.tile_pool(name="p1tmp", bufs=3) as p1t, \
             tc.tile_pool(name="p1s", bufs=4) as p1s, \
             tc.tile_pool(name="ln1", bufs=1) as ln1p, \
             tc.tile_pool(name="p1ps", bufs=4, space="PSUM") as ps1:
            g1_rep = ln1p.tile([P, D], F32)
            nc.gpsimd.dma_start(g1_rep[:], bcast_ap(g1))
            be1_rep = ln1p.tile([P, D], F32)
            nc.gpsimd.dma_start(be1_rep[:], bcast_ap(be1))
            eps_t = ln1p.tile([P, 1], F32)
            nc.vector.memset(eps_t[:], EPS)

            for t in range(ST):
                x_t = p1t.tile([P, D], F32, tag="x_t")
                nc.sync.dma_start(x_t[:], xkv[t * P:(t + 1) * P, :])
                stats = p1s.tile([P, 2, 6], F32, tag="stats")
                xv = x_t[:].rearrange("p (s f) -> p s f", s=2)
                for s in range(2):
                    nc.vector.bn_stats(stats[:, s, :], xv[:, s, :])
                mv = p1s.tile([P, 2], F32, tag="mv")
                nc.vector.bn_aggr(mv[:], stats[:])
                std = p1s.tile([P, 1], F32, tag="std")
                nc.scalar.activation(std[:], mv[:, 1:2], AF.Sqrt, bias=eps_t[:])
                nc.vector.reciprocal(std[:], std[:])
                xn_t = p1t.tile([P, D], F32R, tag="xn_t")
                nc.vector.tensor_scalar(
                    xn_t[:], x_t[:], scalar1=mv[:, 0:1], scalar2=std[:],
                    op0=ALU.subtract, op1=ALU.mult)
                if ln_affine:
                    nc.vector.tensor_tensor(xn_t[:], xn_t[:], g1_rep[:], ALU.mult)
                    nc.vector.tensor_tensor(xn_t[:], xn_t[:], be1_rep[:], ALU.add)
                for j2 in range(KD // 2):
                    pst = ps1.tile([P, 2, P], F32, tag="tp")
                    for h in range(2):
                        nc.tensor.transpose(
                            pst[:, h, :].bitcast(F32R),
                            xn_t[:, (2 * j2 + h) * P:(2 * j2 + h + 1) * P], ident[:])
                    nc.vector.tensor_copy(
                        xn_kvT[:, 2 * j2:2 * j2 + 2, t * P:(t + 1) * P], pst[:])

        if DEBUG:
            nc.sync.dma_start(dbg["xnkvT"], xn_kvT[:])

        # ---- Phase 3: per-group QKV projection + attention ----
        with tc.tile_pool(name="kv", bufs=1) as kvp, \
             tc.tile_pool(name="wst", bufs=2) as wsp, \
             tc.tile_pool(name="expp", bufs=2) as expp, \
             tc.tile_pool(name="qpad", bufs=1) as qpp, \
             tc.tile_pool(name="rcbc", bufs=1) as rcp, \
             tc.tile_pool(name="aps", bufs=1, space="PSUM") as aps:

            zsc = qpp.tile([P, QS], F32)
            nc.vector.memset(zsc[:], 0.0)
            qpadA = [qpp.tile([P, QS], F32R, name=f"qpadA{i}") for i in range(1)]
            qpadB = [qpp.tile([P, QS], F32R, name=f"qpadB{i}") for i in range(1)]
            for i in range(1):
                nc.vector.tensor_copy(qpadA[i][:], zsc[:])
                nc.vector.tensor_copy(qpadB[i][:], zsc[:])

            QT_g = kvp.tile([P, 2, NQ], F32R)
            KT_g = kvp.tile([P, 2, S], F32R)
            # per (toktile, pair, head j): [V_head(64) | 1 | 0(63)]
            V_gp = kvp.tile([P, ST, 2, 2, P], F32R)
            for t in range(ST):
                nc.vector.tensor_copy(
                    V_gp[:, t], zsc[:].rearrange("p (a b m) -> p a b m", a=2, b=2))
            one_r = qpp.tile([P, 1], F32R)
            nc.vector.tensor_copy(one_r[:], ones_f[:, 0:1])
            for t in range(ST):
                for pi in range(2):
                    for j in range(2):
                        nc.vector.tensor_copy(V_gp[:, t, pi, j, 64:65], one_r[:])
            it_count = 0

            for g in range(NG):
                for pl in range(2):   # head pairs 2g, 2g+1
                    pr = 2 * g + pl
                    wq_t = wsp.tile([P, KD, P], F32R, tag="wq_t")
                    nc.sync.dma_start(wq_t[:], Wq[pr])
                    for q in range(NQS):
                        ps = aps.tile([P, QS], F32, tag="pp", bufs=2)
                        for kd in range(KD):
                            nc.tensor.matmul(
                                ps[:], wq_t[:, kd, :], xn_kvT[:, kd, q * QS:(q + 1) * QS],
                                start=(kd == 0), stop=(kd == KD - 1))
                        nc.vector.tensor_scalar_add(
                            QT_g[:, pl, q * QS:(q + 1) * QS], ps[:], bq_t[:, pr:pr + 1])
                    wk_t = wsp.tile([P, KD, P], F32R, tag="wk_t")
                    nc.sync.dma_start(wk_t[:], Wk[pr])
                    for q in range(NKS):
                        ps = aps.tile([P, QS], F32, tag="pp", bufs=2)
                        for kd in range(KD):
                            nc.tensor.matmul(
                                ps[:], wk_t[:, kd, :], xn_kvT[:, kd, q * QS:(q + 1) * QS],
                                start=(kd == 0), stop=(kd == KD - 1))
                        nc.vector.tensor_scalar_add(
                            KT_g[:, pl, q * QS:(q + 1) * QS], ps[:], bk_t[:, pr:pr + 1])
                wv_t = wsp.tile([P, KD, 256], F32R, tag="wv_t", bufs=1)
                nc.sync.dma_start(wv_t[:], Wv[g])
                for t in range(ST):
                    ps = aps.tile([P, QS], F32, tag="pp", bufs=2)
                    for kd in range(KD):
                        nc.tensor.matmul(
                            ps[:, 0:256], xn_kvT[:, kd, t * P:(t + 1) * P], wv_t[:, kd, :],
                            start=(kd == 0), stop=(kd == KD - 1))
                    for pi in range(2):
                        nc.vector.tensor_tensor(
                            V_gp[:, t, pi, :, 0:64],
                            ps[:, pi * 128:(pi + 1) * 128].rearrange("p (j m) -> p j m", j=2),
                            bv_rep[:, g * 256 + pi * 128:g * 256 + (pi + 1) * 128].rearrange(
                                "p (j m) -> p j m", j=2), ALU.add)

                if DEBUG and g == 0:
                    nc.sync.dma_start(dbg["kt0"], KT_g[:])
                    nc.sync.dma_start(dbg["v0"], V_gp[:])
                    nc.sync.dma_start(dbg["qt0"], QT_g[:])

                for q in range(NQS):
                    for pl in range(2):
                        pr = 2 * g + pl
                        i = it_count % 1
                        it_count += 1
                        qsl = slice(q * QS, (q + 1) * QS)
                        nc.vector.tensor_copy(qpadA[i][0:64, :], QT_g[0:64, pl, qsl])
                        nc.vector.tensor_copy(qpadB[i][64:128, :], QT_g[64:128, pl, qsl])
                        av1 = aps.tile([P, QS], F32, tag="av1")
                        av2 = aps.tile([P, QS], F32, tag="av2")
                        for kt in range(ST):
                            ktsl = slice(kt * P, (kt + 1) * P)
                            sAB = aps.tile([P, 2, QS], F32, tag="sAB", bufs=2)
                            nc.tensor.matmul(sAB[:, 0, :], KT_g[:, pl, ktsl], qpadA[i][:],
                                             start=True, stop=True)
                            nc.tensor.matmul(sAB[:, 1, :], KT_g[:, pl, ktsl], qpadB[i][:],
                                             start=True, stop=True)
                            eAB = expp.tile([P, 2, QS], F32R, tag="eAB")
                            nc.scalar.activation(eAB[:], sAB[:], AF.Exp, scale=0.125)
                            eA = eAB[:, 0, :]
                            eB = eAB[:, 1, :]
                            if DEBUG and g == 0 and q == 0 and pl == 0 and kt == 0:
                                nc.sync.dma_start(dbg["e0"], eA)
                                s0c = rcp.tile([P, QS], F32, tag="s0c")
                                nc.vector.tensor_copy(s0c[:], sAB[:, 0, :])
                                nc.sync.dma_start(dbg["s0"], s0c[:])
                            st, sp = (kt == 0), (kt == ST - 1)
                            nc.tensor.matmul(av1[:], V_gp[:, kt, pl, 0, :], eA,
                                             start=st, stop=sp, skip_group_check=True)
                            nc.tensor.matmul(av2[:], V_gp[:, kt, pl, 1, :], eB,
                                             start=st, stop=sp, skip_group_check=True)
                        # free the av psums fast: copy to SBUF, divide from there
                        avc = rcp.tile([65, 2, QS], F32, tag="avc")
                        nc.vector.tensor_copy(avc[0:65, 0, :], av1[0:65, :])
                        nc.vector.tensor_copy(avc[0:65, 1, :], av2[0:65, :])
                        nc.vector.reciprocal(avc[64:65, 0, :], avc[64:65, 0, :])
                        nc.vector.reciprocal(avc[64:65, 1, :], avc[64:65, 1, :])
                        rcd = dramp.tile([2, QS], F32, tag="rcd", bufs=2)
                        nc.sync.dma_start(rcd[0:1, :], avc[64:65, 0, :])
                        nc.sync.dma_start(rcd[1:2, :], avc[64:65, 1, :])
                        bcA = rcp.tile([64, QS], F32, tag="bcA")
                        bcB = rcp.tile([64, QS], F32, tag="bcB")

                        def _b64(row_ap):
                            return bass.AP(tensor=row_ap.tensor, offset=row_ap.offset,
                                           ap=[[0, 64]] + list(row_ap.ap)[1:])

                        nc.sync.dma_start(bcA[:], _b64(rcd[0:1, :]))
                        nc.sync.dma_start(bcB[:], _b64(rcd[1:2, :]))
                        if DEBUG and g == 0 and q == 0 and pl == 0:
                            nc.sync.dma_start(dbg["av0"], avc[:])
                        nc.vector.tensor_tensor(RT[0:64, pr, qsl], avc[0:64, 0, :], bcA[:], ALU.mult)
                        stB = rcp.tile([64, QS], F32R, tag="stB")
                        nc.vector.tensor_tensor(stB[:], avc[0:64, 1, :], bcB[:], ALU.mult)
                        nc.sync.dma_start(RT[64:128, pr, qsl], stB[:])

        xn_es.close()

        if DEBUG:
            nc.sync.dma_start(dbg["rt"], RT[:])

        # ---- Phase 4a: O-projection + residual -> x2 (DRAM) ----
        with tc.tile_pool(name="p4tmp", bufs=2) as p4t, \
             tc.tile_pool(name="p4ps", bufs=2, space="PSUM") as ps4, \
             tc.tile_pool(name="p4tps", bufs=4, space="PSUM") as ps4t:
            for q in range(NQS):
                attnT = p4t.tile([P, KD, QS], F32R, tag="attnT")
                for mt in range(KD):
                    wo_t = p4t.tile([P, KD, P], F32R, tag="wo_t")
                    nc.sync.dma_start(wo_t[:], Wo[mt])
                    ps = ps4.tile([P, QS], F32, tag="pp")
                    for kd in range(KD):
                        nc.tensor.matmul(
                            ps[:], wo_t[:, kd, :], RT[:, kd, q * QS:(q + 1) * QS],
                            start=(kd == 0), stop=(kd == KD - 1))
                    nc.vector.tensor_scalar_add(
                        attnT[:, mt, :], ps[:], bo_t[:, mt:mt + 1])
                for j in range(QS // P):
                    tt = q * (QS // P) + j
                    xr_t = p4t.tile([P, D], F32, tag="xr_t")
                    nc.sync.dma_start(xr_t[:], xkv[tt * P:(tt + 1) * P, :])
                    x2_t = p4t.tile([P, D], F32, tag="x2_t")
                    for mt in range(KD):
                        pst = ps4t.tile([P, P], F32, tag="tp")
                        nc.tensor.transpose(pst[:].bitcast(F32R),
                                            attnT[:, mt, j * P:(j + 1) * P], ident[:])
                        nc.vector.tensor_tensor(
                            x2_t[:, mt * P:(mt + 1) * P], pst[:],
                            xr_t[:, mt * P:(mt + 1) * P], ALU.add)
                    nc.sync.dma_start(x2d[:, tt, :], x2_t[:])
                    if DEBUG:
                        nc.sync.dma_start(dbg["x2"][:, tt, :], x2_t[:])
        rt_es.close()

        # ---- Phase 4b: LN2 -> xn2T ----
        xn2_es = ExitStack()
        xn2p = xn2_es.enter_context(tc.tile_pool(name="xn2", bufs=1))
        xn2T = xn2p.tile([P, KD, NQ], F32R)
        with tc.tile_pool(name="p4btmp", bufs=3) as p4bt, \
             tc.tile_pool(name="p4bs", bufs=4) as p4bs, \
             tc.tile_pool(name="ln2", bufs=1) as ln2p, \
             tc.tile_pool(name="p4bps", bufs=4, space="PSUM") as ps4b:
            g2_rep = ln2p.tile([P, D], F32)
            nc.gpsimd.dma_start(g2_rep[:], bcast_ap(g2))
            be2_rep = ln2p.tile([P, D], F32)
            nc.gpsimd.dma_start(be2_rep[:], bcast_ap(be2))
            eps2_t = ln2p.tile([P, 1], F32)
            nc.vector.memset(eps2_t[:], EPS)

            for tt in range(QTT):
                x2_t = p4bt.tile([P, D], F32, tag="x2_t")
                nc.sync.dma_start(x2_t[:], x2d[:, tt, :])
                stats = p4bs.tile([P, 2, 6], F32, tag="stats2")
                xv = x2_t[:].rearrange("p (s f) -> p s f", s=2)
                for s in range(2):
                    nc.vector.bn_stats(stats[:, s, :], xv[:, s, :])
                mv = p4bs.tile([P, 2], F32, tag="mv2")
                nc.vector.bn_aggr(mv[:], stats[:])
                std = p4bs.tile([P, 1], F32, tag="std2")
                nc.scalar.activation(std[:], mv[:, 1:2], AF.Sqrt, bias=eps2_t[:])
                nc.vector.reciprocal(std[:], std[:])
                xn2_t = p4bt.tile([P, D], F32R, tag="xn2_t")
                nc.vector.tensor_scalar(
                    xn2_t[:], x2_t[:], scalar1=mv[:, 0:1], scalar2=std[:],
                    op0=ALU.subtract, op1=ALU.mult)
                if ln_affine:
                    nc.vector.tensor_tensor(xn2_t[:], xn2_t[:], g2_rep[:], ALU.mult)
                    nc.vector.tensor_tensor(xn2_t[:], xn2_t[:], be2_rep[:], ALU.add)
                for j2 in range(KD // 2):
                    pst = ps4b.tile([P, 2, P], F32, tag="tp")
                    for h in range(2):
                        nc.tensor.transpose(
                            pst[:, h, :].bitcast(F32R),
                            xn2_t[:, (2 * j2 + h) * P:(2 * j2 + h + 1) * P], ident[:])
                    nc.vector.tensor_copy(
                        xn2T[:, 2 * j2:2 * j2 + 2, tt * P:(tt + 1) * P], pst[:])

        # ---- Phase 5: MLP (h1 in bf16, single full-width token pass) ----
        with tc.tile_pool(name="p5tmp", bufs=3) as p5t, \
             tc.tile_pool(name="h1", bufs=1) as h1p, \
             tc.tile_pool(name="w2st", bufs=2) as w2p, \
             tc.tile_pool(name="p5ps", bufs=2, space="PSUM") as ps5, \
             tc.tile_pool(name="p5tps", bufs=4, space="PSUM") as ps5t:
            mdt = BF16 if mlp_bf16 else F32R
            n_hslice = 1 if mlp_bf16 else NQS
            HW_ = NQ // n_hslice
            out_acc = h1p.tile([P, QTT, D], F32)
            for hs in range(n_hslice):
                h1T = h1p.tile([P, FT, HW_], mdt, tag="h1T")
                for ft in range(FT):
                    w1_t = p5t.tile([P, KD, P], F32R, tag="w1_t")
                    nc.sync.dma_start(w1_t[:], W1[ft])
                    for sl in range(HW_ // QS):
                        ssl = slice(hs * HW_ + sl * QS, hs * HW_ + (sl + 1) * QS)
                        ps = ps5.tile([P, QS], F32, tag="pp")
                        for kd in range(KD):
                            nc.tensor.matmul(
                                ps[:], w1_t[:, kd, :], xn2T[:, kd, ssl],
                                start=(kd == 0), stop=(kd == KD - 1))
                        nc.scalar.activation(h1T[:, ft, sl * QS:(sl + 1) * QS], ps[:],
                                             AF.Gelu, bias=b1_t[:, ft:ft + 1])
                for mt in range(KD):
                    w2_t = w2p.tile([P, FT, P], mdt, tag="w2_t")
                    nc.sync.dma_start(w2_t[:], W2[mt])
                    for sl in range(HW_ // QS):
                        ssl_loc = slice(sl * QS, (sl + 1) * QS)
                        ps = ps5.tile([P, QS], F32, tag="pp")
                        for ft in range(FT):
                            nc.tensor.matmul(
                                ps[:], w2_t[:, ft, :], h1T[:, ft, ssl_loc],
                                start=(ft == 0), stop=(ft == FT - 1))
                        outT = p5t.tile([P, QS], F32R, tag="outT", bufs=2)
                        nc.vector.tensor_scalar_add(outT[:], ps[:], b2_t[:, mt:mt + 1])
                        for j in range(QS // P):
                            tt = hs * (HW_ // P) + sl * (QS // P) + j
                            pst = ps5t.tile([P, P], F32, tag="tp")
                            nc.tensor.transpose(pst[:].bitcast(F32R),
                                                outT[:, j * P:(j + 1) * P], ident[:])
                            nc.vector.tensor_copy(out_acc[:, tt, mt * P:(mt + 1) * P], pst[:])
            for tt in range(QTT):
                x2_t = p5t.tile([P, D], F32, tag="x2r_t")
                nc.sync.dma_start(x2_t[:], x2d[:, tt, :])
                ob = p5t.tile([P, D], F32, tag="ob")
                nc.vector.tensor_tensor(ob[:], out_acc[:, tt, :], x2_t[:], ALU.add)
                nc.sync.dma_start(out[tt * P:(tt + 1) * P, :], ob[:])

        xn2_es.close()
        es.close()

    nc.compile()
    return nc


def kernel(**inputs):
    inputs = {k: np.ascontiguousarray(np.asarray(v), dtype=np.float32)
              for k, v in inputs.items()}
    ln_affine = not (
        np.all(inputs["ln1_g"] == 1.0) and np.all(inputs["ln1_b"] == 0.0)
        and np.all(inputs["ln2_g"] == 1.0) and np.all(inputs["ln2_b"] == 0.0))
    key = ("nc", ln_affine, MLP_BF16)
    if key not in _CACHE:
        _CACHE[key] = _build(ln_affine=ln_affine, mlp_bf16=MLP_BF16)
    nc = _CACHE[key]

    x = inputs["x"]
    def tile_w(W, n_out, m):
        # [Din, Dout] -> [Dout/m, 128, Din/128, m]
        Din, Dout = W.shape
        return np.ascontiguousarray(
            W.reshape(Din // P, P, n_out, m).transpose(2, 1, 0, 3))

    shared = {
        "Wq": tile_w(inputs["Wq"], KD, P), "Wk": tile_w(inputs["Wk"], KD, P),
        "Wv": tile_w(inputs["Wv"], NG, 256), "Wo": tile_w(inputs["Wo"], KD, P),
        "W1": tile_w(inputs["W1"], FT, P),
        "W2": (tile_w(inputs["W2"], KD, P).astype(__import__("ml_dtypes").bfloat16)
               if MLP_BF16 else tile_w(inputs["W2"], KD, P)),
        "bq": inputs["bq"], "bk": inputs["bk"], "bv": inputs["bv"], "bo": inputs["bo"],
        "b1": inputs["b1"], "b2": inputs["b2"],
        "g1": inputs["ln1_g"], "be1": inputs["ln1_b"],
        "g2": inputs["ln2_g"], "be2": inputs["ln2_b"],
    }
    in_maps = []
    for c in range(8):
        b, half = c // 2, c % 2
        m = dict(shared)
        # query half first; attention is permutation-invariant over kv order
        m["xkv"] = np.ascontiguousarray(
            np.concatenate([x[b, half * NQ:(half + 1) * NQ, :],
                            x[b, (1 - half) * NQ:(2 - half) * NQ, :]], axis=0))
        in_maps.append(m)

    trace = bool(int(os.environ.get("KERNEL_TRACE", "0")))
    kw = {}
    if trace:
        kw = dict(trace=True, tmpdir=os.environ.get("KERNEL_TRACE_DIR") or None)
    res = bass_utils.run_bass_kernel_spmd(nc, in_maps, core_ids=list(range(8)), **kw)
    _CACHE["last_results"] = res
    _CACHE["nc"] = nc
    _CACHE["last_in_maps"] = in_maps

    outa = np.empty((B, S, D), dtype=np.float32)
    for c in range(8):
        b, half = c // 2, c % 2
        outa[b, half * NQ:(half + 1) * NQ, :] = res.results[c]["out"]
    return outa


# revision 30
# speedup vs baseline: 1.0118x; 1.0014x over previous
"""Trainium2 Bass kernel for a dense transformer encoder block (B=4, S=2048,
D=1024, H=16, MLP=4096).

Sharding: 8 cores = 4 batch elements x 2 query-halves, no collectives. Each
core's kv sequence is host-reordered so its 1024 query tokens come first
(attention is permutation-invariant over keys), so Q/residual tensors are
plain slices of the kv set. K/V are computed for the full 2048-token sequence
(~6% duplicated FLOPs vs. perfect sharding).

Per-core dataflow is feature-major ("T" = [feature, token]) so every matmul
has contraction dim 128 on partitions (sub-128-contraction matmuls fail to
load on this stack, all dtypes):
  LN1 (token-major, bn_stats) -> PE-transpose -> xnT            [phase 1]
  per head-group of 4 heads: Q/K/V projections from xnT         [phase 3]
    scores^T = KT_pair^T @ Qpad   (zero-padded rhs selects one head
                                   of the packed pair; K=128 kept)
    exp on ACT, scale=1/8 fused, both heads in one [128,1024] op -> f32r
    AV+den fused: lhsT = [V_head | 1 | 0] so psum rows 0:64 = V^T e and
      row 64 = sum(e); one augmented matmul per (ktile, head)
    reciprocal of row 64, partition-broadcast via DRAM round-trip DMA
      (stride-0 partition APs are DRAM-only), multiply -> RT; head B's
      rows shift 0:64 -> 64:128 via a small SBUF->SBUF DMA
  O-proj +bo, PE-transpose back, +residual -> x2 -> DRAM        [phase 4a]
  LN2 on x2 -> PE-transpose -> xn2T                             [phase 4b]
  MLP: h1 (+b1 and exact-erf Gelu fused on ACT), h2 (+b2),      [phase 5]
    PE-transpose back, +x2 residual -> out

Numerics: matmuls in float32r (TF32-class, ~1.5e-4 rel err, full PE rate at
free-dim >= 256; requires producers typed f32r), fp32 PSUM accumulation,
fp32 layernorm/softmax scalars. End-to-end rel err ~1.4e-4.

Weights are host-retiled to [tile, partition, kd, m] so each weight-tile DMA
is one contiguous block (4KB per-partition chunks). LN affine (g=1, b=0 for
this problem's inputs) is skipped at build time when the host detects
identity values; a full-affine variant is built otherwise.

Cost-model (TimelineSim) span: ~867 us/core; PE busy ~820 us (the binding
engine; attention runs at 50% array utilization, the price of the K=128
constraint with DH=64 heads and no working sub-128 row/col tiling).
"""

import os
import sys

sys.path.insert(0, "/opt/trn_rl_repo")

from contextlib import ExitStack

import numpy as np

import concourse.bass as bass
import concourse.tile as tile
from concourse import bacc, bass_utils, mybir
from concourse.masks import make_identity

F32 = mybir.dt.float32
F32R = mybir.dt.float32r
BF16 = mybir.dt.bfloat16
AF = mybir.ActivationFunctionType
ALU = mybir.AluOpType

B, S, D = 4, 2048, 1024
H, DH, MLP = 16, 64, 4096
P = 128
KD = D // P            # 8 partition tiles over D
FT = MLP // P          # 32 partition tiles over MLP dim
NQ = S // 2            # 1024 query tokens per core
ST = S // P            # 16 kv token tiles
QTT = NQ // P          # 8 q token tiles
QS = 512               # free-dim slice
NQS = NQ // QS         # 2
NKS = S // QS          # 4
NG = 4                 # head groups
EPS = 1e-6
DEBUG = bool(int(os.environ.get("KERNEL_DEBUG", "0")))
MLP_BF16 = bool(int(os.environ.get("KERNEL_MLP_BF16", "0")))

_CACHE = {}


def _build(ln_affine=True, mlp_bf16=True):
    nc = bacc.Bacc(None, target_bir_lowering=False, debug=False, num_devices=8)

    xkv = nc.dram_tensor("xkv", [S, D], F32, kind="ExternalInput").ap()
    # weights arrive host-tiled: [tile, p, kd, m] so each SBUF weight tile is
    # one contiguous DRAM block (4KB+ per-partition DMA chunks)
    Wq = nc.dram_tensor("Wq", [KD, P, KD, P], F32R, kind="ExternalInput").ap()
    Wk = nc.dram_tensor("Wk", [KD, P, KD, P], F32R, kind="ExternalInput").ap()
    Wv = nc.dram_tensor("Wv", [NG, P, KD, 256], F32R, kind="ExternalInput").ap()
    Wo = nc.dram_tensor("Wo", [KD, P, KD, P], F32R, kind="ExternalInput").ap()
    W1 = nc.dram_tensor("W1", [FT, P, KD, P], F32R, kind="ExternalInput").ap()
    W2 = nc.dram_tensor("W2", [KD, P, FT, P], BF16 if mlp_bf16 else F32R, kind="ExternalInput").ap()
    bq = nc.dram_tensor("bq", [D], F32, kind="ExternalInput").ap()
    bk = nc.dram_tensor("bk", [D], F32, kind="ExternalInput").ap()
    bv = nc.dram_tensor("bv", [D], F32, kind="ExternalInput").ap()
    bo = nc.dram_tensor("bo", [D], F32, kind="ExternalInput").ap()
    b1 = nc.dram_tensor("b1", [MLP], F32, kind="ExternalInput").ap()
    b2 = nc.dram_tensor("b2", [D], F32, kind="ExternalInput").ap()
    g1 = nc.dram_tensor("g1", [D], F32, kind="ExternalInput").ap()
    be1 = nc.dram_tensor("be1", [D], F32, kind="ExternalInput").ap()
    g2 = nc.dram_tensor("g2", [D], F32, kind="ExternalInput").ap()
    be2 = nc.dram_tensor("be2", [D], F32, kind="ExternalInput").ap()
    out = nc.dram_tensor("out", [NQ, D], F32, kind="ExternalOutput").ap()

    dbg = {}
    if DEBUG:
        dbg["xnkvT"] = nc.dram_tensor("d_xnkvT", [P, KD, S], F32R, kind="ExternalOutput").ap()
        dbg["qt0"] = nc.dram_tensor("d_qt0", [P, 2, NQ], F32R, kind="ExternalOutput").ap()
        dbg["kt0"] = nc.dram_tensor("d_kt0", [P, 2, S], F32R, kind="ExternalOutput").ap()
        dbg["v0"] = nc.dram_tensor("d_v0", [P, ST, 2, 2, P], F32R, kind="ExternalOutput").ap()
        dbg["rt"] = nc.dram_tensor("d_rt", [P, KD, NQ], F32R, kind="ExternalOutput").ap()
        dbg["e0"] = nc.dram_tensor("d_e0", [P, QS], F32R, kind="ExternalOutput").ap()
        dbg["s0"] = nc.dram_tensor("d_s0", [P, QS], F32, kind="ExternalOutput").ap()
        dbg["av0"] = nc.dram_tensor("d_av0", [65, 2, QS], F32, kind="ExternalOutput").ap()
        dbg["x2"] = nc.dram_tensor("d_x2", [P, QTT, D], F32, kind="ExternalOutput").ap()

    def bcast_ap(vec):
        # [D] dram vector -> [128, D] partition-replicated DMA source
        return bass.AP(tensor=vec.tensor, offset=vec.offset, ap=[[0, P]] + list(vec.ap))



    with tile.TileContext(nc) as tc:
        es = ExitStack()
        params = es.enter_context(tc.tile_pool(name="params", bufs=1))
        dramp = es.enter_context(tc.tile_pool(name="dram", bufs=1, space="DRAM"))
        x2d = dramp.tile([P, QTT, D], F32)

        ident_f = params.tile([P, P], F32)
        make_identity(nc, ident_f)
        ident = params.tile([P, P], F32R)
        nc.vector.tensor_copy(ident[:], ident_f[:])
        ones_f = params.tile([P, 1], F32)
        nc.vector.memset(ones_f[:, 0:1], 1.0)

        def pvec(v, n, nm):  # [n*128] -> [128, n] (dim o*128+p -> [p, o])
            t = params.tile([P, n], F32, name=nm)
            nc.sync.dma_start(t[:], v.rearrange("(o p) -> p o", p=P))
            return t

        bq_t = pvec(bq, KD, "bq_t")
        bk_t = pvec(bk, KD, "bk_t")
        bo_t = pvec(bo, KD, "bo_t")
        b2_t = pvec(b2, KD, "b2_t")
        b1_t = pvec(b1, FT, "b1_t")
        bv_rep = params.tile([P, D], F32)
        nc.gpsimd.dma_start(bv_rep[:], bcast_ap(bv))

        rt_es = ExitStack()
        rtp = rt_es.enter_context(tc.tile_pool(name="rt", bufs=1))
        RT = rtp.tile([P, KD, NQ], F32R)

        xn_es = ExitStack()
        xnp = xn_es.enter_context(tc.tile_pool(name="xn", bufs=1))
        xn_kvT = xnp.tile([P, KD, S], F32R)

        # ---- Phase 1: LN1 + transpose to feature-major ----
        with tc.tile_pool(name="p1tmp", bufs=3) as p1t, \
             tc.tile_pool(name="p1s", bufs=4) as p1s, \
             tc.tile_pool(name="ln1", bufs=1) as ln1p, \
             tc.tile_pool(name="p1ps", bufs=4, space="PSUM") as ps1:
            g1_rep = ln1p.tile([P, D], F32)
            nc.gpsimd.dma_start(g1_rep[:], bcast_ap(g1))
            be1_rep = ln1p.tile([P, D], F32)
            nc.gpsimd.dma_start(be1_rep[:], bcast_ap(be1))
            eps_t = ln1p.tile([P, 1], F32)
            nc.vector.memset(eps_t[:], EPS)

            for t in range(ST):
                x_t = p1t.tile([P, D], F32, tag="x_t")
                nc.sync.dma_start(x_t[:], xkv[t * P:(t + 1) * P, :])
                stats = p1s.tile([P, 2, 6], F32, tag="stats")
                xv = x_t[:].rearrange("p (s f) -> p s f", s=2)
                for s in range(2):
                    nc.vector.bn_stats(stats[:, s, :], xv[:, s, :])
                mv = p1s.tile([P, 2], F32, tag="mv")
                nc.vector.bn_aggr(mv[:], stats[:])
                std = p1s.tile([P, 1], F32, tag="std")
                nc.scalar.activation(std[:], mv[:, 1:2], AF.Sqrt, bias=eps_t[:])
                nc.vector.reciprocal(std[:], std[:])
                xn_t = p1t.tile([P, D], F32R, tag="xn_t")
                nc.vector.tensor_scalar(
                    xn_t[:], x_t[:], scalar1=mv[:, 0:1], scalar2=std[:],
                    op0=ALU.subtract, op1=ALU.mult)
                if ln_affine:
                    nc.vector.tensor_tensor(xn_t[:], xn_t[:], g1_rep[:], ALU.mult)
                    nc.vector.tensor_tensor(xn_t[:], xn_t[:], be1_rep[:], ALU.add)
                for j2 in range(KD // 2):
                    pst = ps1.tile([P, 2, P], F32, tag="tp")
                    for h in range(2):
                        nc.tensor.transpose(
                            pst[:, h, :].bitcast(F32R),
                            xn_t[:, (2 * j2 + h) * P:(2 * j2 + h + 1) * P], ident[:])
                    nc.vector.tensor_copy(
                        xn_kvT[:, 2 * j2:2 * j2 + 2, t * P:(t + 1) * P], pst[:])

        if DEBUG:
            nc.sync.dma_start(dbg["xnkvT"], xn_kvT[:])

        # ---- Phase 3: per-group QKV projection + attention ----
        with tc.tile_pool(name="kv", bufs=1) as kvp, \
             tc.tile_pool(name="wst", bufs=2) as wsp, \
             tc.tile_pool(name="expp", bufs=2) as expp, \
             tc.tile_pool(name="qpad", bufs=1) as qpp, \
             tc.tile_pool(name="rcbc", bufs=1) as rcp, \
             tc.tile_pool(name="aps", bufs=1, space="PSUM") as aps:

            zsc = qpp.tile([P, QS], F32)
            nc.vector.memset(zsc[:], 0.0)
            qpadA = [qpp.tile([P, QS], F32R, name=f"qpadA{i}") for i in range(1)]
            qpadB = [qpp.tile([P, QS], F32R, name=f"qpadB{i}") for i in range(1)]
            for i in range(1):
                nc.vector.tensor_copy(qpadA[i][:], zsc[:])
                nc.vector.tensor_copy(qpadB[i][:], zsc[:])

            QT_g = kvp.tile([P, 2, NQ], F32R)
            KT_g = kvp.tile([P, 2, S], F32R)
            # per (toktile, pair, head j): [V_head(64) | 1 | 0(63)]
            V_gp = kvp.tile([P, ST, 2, 2, P], F32R)
            for t in range(ST):
                nc.vector.tensor_copy(
                    V_gp[:, t], zsc[:].rearrange("p (a b m) -> p a b m", a=2, b=2))
            one_r = qpp.tile([P, 1], F32R)
            nc.vector.tensor_copy(one_r[:], ones_f[:, 0:1])
            for t in range(ST):
                for pi in range(2):
                    for j in range(2):
                        nc.vector.tensor_copy(V_gp[:, t, pi, j, 64:65], one_r[:])
            it_count = 0

            for g in range(NG):
                for pl in range(2):   # head pairs 2g, 2g+1
                    pr = 2 * g + pl
                    wq_t = wsp.tile([P, KD, P], F32R, tag="wq_t")
                    nc.sync.dma_start(wq_t[:], Wq[pr])
                    for q in range(NQS):
                        ps = aps.tile([P, QS], F32, tag="pp", bufs=2)
                        for kd in range(KD):
                            nc.tensor.matmul(
                                ps[:], wq_t[:, kd, :], xn_kvT[:, kd, q * QS:(q + 1) * QS],
                                start=(kd == 0), stop=(kd == KD - 1))
                        nc.vector.tensor_scalar_add(
                            QT_g[:, pl, q * QS:(q + 1) * QS], ps[:], bq_t[:, pr:pr + 1])
                    wk_t = wsp.tile([P, KD, P], F32R, tag="wk_t")
                    nc.sync.dma_start(wk_t[:], Wk[pr])
                    for q in range(NKS):
                        ps = aps.tile([P, QS], F32, tag="pp", bufs=2)
                        for kd in range(KD):
                            nc.tensor.matmul(
                                ps[:], wk_t[:, kd, :], xn_kvT[:, kd, q * QS:(q + 1) * QS],
                                start=(kd == 0), stop=(kd == KD - 1))
                        nc.vector.tensor_scalar_add(
                            KT_g[:, pl, q * QS:(q + 1) * QS], ps[:], bk_t[:, pr:pr + 1])
                wv_t = wsp.tile([P, KD, 256], F32R, tag="wv_t", bufs=1)
                nc.sync.dma_start(wv_t[:], Wv[g])
                for t in range(ST):
                    ps = aps.tile([P, QS], F32, tag="pp", bufs=2)
                    for kd in range(KD):
                        nc.tensor.matmul(
                            ps[:, 0:256], xn_kvT[:, kd, t * P:(t + 1) * P], wv_t[:, kd, :],
                            start=(kd == 0), stop=(kd == KD - 1))
                    for pi in range(2):
                        nc.vector.tensor_tensor(
                            V_gp[:, t, pi, :, 0:64],
                            ps[:, pi * 128:(pi + 1) * 128].rearrange("p (j m) -> p j m", j=2),
                            bv_rep[:, g * 256 + pi * 128:g * 256 + (pi + 1) * 128].rearrange(
                                "p (j m) -> p j m", j=2), ALU.add)

                if DEBUG and g == 0:
                    nc.sync.dma_start(dbg["kt0"], KT_g[:])
                    nc.sync.dma_start(dbg["v0"], V_gp[:])
                    nc.sync.dma_start(dbg["qt0"], QT_g[:])

                for q in range(NQS):
                    for pl in range(2):
                        pr = 2 * g + pl
                        i = it_count % 1
                        it_count += 1
                        qsl = slice(q * QS, (q + 1) * QS)
                        nc.vector.tensor_copy(qpadA[i][0:64, :], QT_g[0:64, pl, qsl])
                        nc.vector.tensor_copy(qpadB[i][64:128, :], QT_g[64:128, pl, qsl])
                        av1 = aps.tile([P, QS], F32, tag="av1")
                        av2 = aps.tile([P, QS], F32, tag="av2")
                        for kt in range(ST):
                            ktsl = slice(kt * P, (kt + 1) * P)
                            sAB = aps.tile([P, 2, QS], F32, tag="sAB", bufs=2)
                            nc.tensor.matmul(sAB[:, 0, :], KT_g[:, pl, ktsl], qpadA[i][:],
                                             start=True, stop=True)
                            nc.tensor.matmul(sAB[:, 1, :], KT_g[:, pl, ktsl], qpadB[i][:],
                                             start=True, stop=True)
                            eAB = expp.tile([P, 2, QS], F32R, tag="eAB")
                            nc.scalar.activation(eAB[:], sAB[:], AF.Exp, scale=0.125)
                            eA = eAB[:, 0, :]
                            eB = eAB[:, 1, :]
                            if DEBUG and g == 0 and q == 0 and pl == 0 and kt == 0:
                                nc.sync.dma_start(dbg["e0"], eA)
                                s0c = rcp.tile([P, QS], F32, tag="s0c")
                                nc.vector.tensor_copy(s0c[:], sAB[:, 0, :])
                                nc.sync.dma_start(dbg["s0"], s0c[:])
                            st, sp = (kt == 0), (kt == ST - 1)
                            nc.tensor.matmul(av1[:], V_gp[:, kt, pl, 0, :], eA,
                                             start=st, stop=sp, skip_group_check=True)
                            nc.tensor.matmul(av2[:], V_gp[:, kt, pl, 1, :], eB,
                                             start=st, stop=sp, skip_group_check=True)
                        # free the av psums fast: copy to SBUF, divide from there
                        avc = rcp.tile([65, 2, QS], F32, tag="avc")
                        nc.vector.tensor_copy(avc[0:65, 0, :], av1[0:65, :])
                        nc.vector.tensor_copy(avc[0:65, 1, :], av2[0:65, :])
                        nc.vector.reciprocal(avc[64:65, 0, :], avc[64:65, 0, :])
                        nc.vector.reciprocal(avc[64:65, 1, :], avc[64:65, 1, :])
                        rcd = dramp.tile([2, QS], F32, tag="rcd", bufs=2)
                        nc.sync.dma_start(rcd[0:1, :], avc[64:65, 0, :])
                        nc.sync.dma_start(rcd[1:2, :], avc[64:65, 1, :])
                        bcA = rcp.tile([64, QS], F32, tag="bcA")
                        bcB = rcp.tile([64, QS], F32, tag="bcB")

                        def _b64(row_ap):
                            return bass.AP(tensor=row_ap.tensor, offset=row_ap.offset,
                                           ap=[[0, 64]] + list(row_ap.ap)[1:])

                        nc.sync.dma_start(bcA[:], _b64(rcd[0:1, :]))
                        nc.sync.dma_start(bcB[:], _b64(rcd[1:2, :]))
                        if DEBUG and g == 0 and q == 0 and pl == 0:
                            nc.sync.dma_start(dbg["av0"], avc[:])
                        nc.vector.tensor_tensor(RT[0:64, pr, qsl], avc[0:64, 0, :], bcA[:], ALU.mult)
                        stB = rcp.tile([64, QS], F32R, tag="stB")
                        nc.vector.tensor_tensor(stB[:], avc[0:64, 1, :], bcB[:], ALU.mult)
                        nc.sync.dma_start(RT[64:128, pr, qsl], stB[:])

        xn_es.close()

        if DEBUG:
            nc.sync.dma_start(dbg["rt"], RT[:])

        # ---- Phase 4a: O-projection + residual -> x2 (DRAM) ----
        with tc.tile_pool(name="p4tmp", bufs=2) as p4t, \
             tc.tile_pool(name="p4ps", bufs=2, space="PSUM") as ps4, \
             tc.tile_pool(name="p4tps", bufs=4, space="PSUM") as ps4t:
            for q in range(NQS):
                attnT = p4t.tile([P, KD, QS], F32R, tag="attnT")
                for mt in range(KD):
                    wo_t = p4t.tile([P, KD, P], F32R, tag="wo_t")
                    nc.sync.dma_start(wo_t[:], Wo[mt])
                    ps = ps4.tile([P, QS], F32, tag="pp")
                    for kd in range(KD):
                        nc.tensor.matmul(
                            ps[:], wo_t[:, kd, :], RT[:, kd, q * QS:(q + 1) * QS],
                            start=(kd == 0), stop=(kd == KD - 1))
                    nc.vector.tensor_scalar_add(
                        attnT[:, mt, :], ps[:], bo_t[:, mt:mt + 1])
                for j in range(QS // P):
                    tt = q * (QS // P) + j
                    xr_t = p4t.tile([P, D], F32, tag="xr_t")
                    nc.sync.dma_start(xr_t[:], xkv[tt * P:(tt + 1) * P, :])
                    x2_t = p4t.tile([P, D], F32, tag="x2_t")
                    for m2 in range(KD // 2):
                        pst = ps4t.tile([P, 2, P], F32, tag="tp")
                        for h in range(2):
                            nc.tensor.transpose(
                                pst[:, h, :].bitcast(F32R),
                                attnT[:, 2 * m2 + h, j * P:(j + 1) * P], ident[:])
                        nc.vector.tensor_tensor(
                            x2_t[:, 2 * m2 * P:(2 * m2 + 2) * P],
                            pst[:].rearrange("p a m -> p (a m)"),
                            xr_t[:, 2 * m2 * P:(2 * m2 + 2) * P], ALU.add)
                    nc.sync.dma_start(x2d[:, tt, :], x2_t[:])
                    if DEBUG:
                        nc.sync.dma_start(dbg["x2"][:, tt, :], x2_t[:])
        rt_es.close()

        # ---- Phase 4b: LN2 -> xn2T ----
        xn2_es = ExitStack()
        xn2p = xn2_es.enter_context(tc.tile_pool(name="xn2", bufs=1))
        xn2T_h = [xn2p.tile([P, KD, QS], F32R, name=f"xn2T{h}") for h in range(NQS)]
        with tc.tile_pool(name="p4btmp", bufs=3) as p4bt, \
             tc.tile_pool(name="p4bs", bufs=4) as p4bs, \
             tc.tile_pool(name="ln2", bufs=1) as ln2p, \
             tc.tile_pool(name="p4bps", bufs=4, space="PSUM") as ps4b:
            g2_rep = ln2p.tile([P, D], F32)
            nc.gpsimd.dma_start(g2_rep[:], bcast_ap(g2))
            be2_rep = ln2p.tile([P, D], F32)
            nc.gpsimd.dma_start(be2_rep[:], bcast_ap(be2))
            eps2_t = ln2p.tile([P, 1], F32)
            nc.vector.memset(eps2_t[:], EPS)

            for tt in range(QTT):
                x2_t = p4bt.tile([P, D], F32, tag="x2_t")
                nc.sync.dma_start(x2_t[:], x2d[:, tt, :])
                stats = p4bs.tile([P, 2, 6], F32, tag="stats2")
                xv = x2_t[:].rearrange("p (s f) -> p s f", s=2)
                for s in range(2):
                    nc.vector.bn_stats(stats[:, s, :], xv[:, s, :])
                mv = p4bs.tile([P, 2], F32, tag="mv2")
                nc.vector.bn_aggr(mv[:], stats[:])
                std = p4bs.tile([P, 1], F32, tag="std2")
                nc.scalar.activation(std[:], mv[:, 1:2], AF.Sqrt, bias=eps2_t[:])
                nc.vector.reciprocal(std[:], std[:])
                xn2_t = p4bt.tile([P, D], F32R, tag="xn2_t")
                nc.vector.tensor_scalar(
                    xn2_t[:], x2_t[:], scalar1=mv[:, 0:1], scalar2=std[:],
                    op0=ALU.subtract, op1=ALU.mult)
                if ln_affine:
                    nc.vector.tensor_tensor(xn2_t[:], xn2_t[:], g2_rep[:], ALU.mult)
                    nc.vector.tensor_tensor(xn2_t[:], xn2_t[:], be2_rep[:], ALU.add)
                hs_i, loc = tt // (QS // P), (tt % (QS // P)) * P
                for j2 in range(KD // 2):
                    pst = ps4b.tile([P, 2, P], F32, tag="tp")
                    for h in range(2):
                        nc.tensor.transpose(
                            pst[:, h, :].bitcast(F32R),
                            xn2_t[:, (2 * j2 + h) * P:(2 * j2 + h + 1) * P], ident[:])
                    nc.vector.tensor_copy(
                        xn2T_h[hs_i][:, 2 * j2:2 * j2 + 2, loc:loc + P], pst[:])

        # ---- Phase 5: MLP (h1 in bf16, single full-width token pass) ----
        with tc.tile_pool(name="p5tmp", bufs=3) as p5t, \
             tc.tile_pool(name="h1", bufs=1) as h1p, \
             tc.tile_pool(name="w2st", bufs=2) as w2p, \
             tc.tile_pool(name="p5ps", bufs=2, space="PSUM") as ps5, \
             tc.tile_pool(name="p5tps", bufs=4, space="PSUM") as ps5t:
            mdt = BF16 if mlp_bf16 else F32R
            n_hslice = 1 if mlp_bf16 else NQS
            HW_ = NQ // n_hslice
            out_acc = h1p.tile([P, QTT, D], F32)
            for hs in range(n_hslice):
                h1T = h1p.tile([P, FT, HW_], mdt, tag="h1T")
                for ft in range(FT):
                    w1_t = p5t.tile([P, KD, P], F32R, tag="w1_t")
                    nc.sync.dma_start(w1_t[:], W1[ft])
                    for sl in range(HW_ // QS):
                        gsl = (hs * HW_ + sl * QS) // QS
                        ps = ps5.tile([P, QS], F32, tag="pp")
                        for kd in range(KD):
                            nc.tensor.matmul(
                                ps[:], w1_t[:, kd, :], xn2T_h[gsl][:, kd, :],
                                start=(kd == 0), stop=(kd == KD - 1))
                        nc.scalar.activation(h1T[:, ft, sl * QS:(sl + 1) * QS], ps[:],
                                             AF.Gelu, bias=b1_t[:, ft:ft + 1])
                for mt in range(KD):
                    w2_t = w2p.tile([P, FT, P], mdt, tag="w2_t")
                    nc.sync.dma_start(w2_t[:], W2[mt])
                    for sl in range(HW_ // QS):
                        ssl_loc = slice(sl * QS, (sl + 1) * QS)
                        ps = ps5.tile([P, QS], F32, tag="pp")
                        for ft in range(FT):
                            nc.tensor.matmul(
                                ps[:], w2_t[:, ft, :], h1T[:, ft, ssl_loc],
                                start=(ft == 0), stop=(ft == FT - 1))
                        outT = p5t.tile([P, QS], F32R, tag="outT", bufs=2)
                        nc.vector.tensor_scalar_add(outT[:], ps[:], b2_t[:, mt:mt + 1])
                        for j in range(QS // P):
                            tt = hs * (HW_ // P) + sl * (QS // P) + j
                            pst = ps5t.tile([P, P], F32, tag="tp")
                            nc.tensor.transpose(pst[:].bitcast(F32R),
                                                outT[:, j * P:(j + 1) * P], ident[:])
                            nc.vector.tensor_copy(out_acc[:, tt, mt * P:(mt + 1) * P], pst[:])
            for tt in range(QTT):
                x2_t = p5t.tile([P, D], F32, tag="x2r_t")
                nc.sync.dma_start(x2_t[:], x2d[:, tt, :])
                ob = p5t.tile([P, D], F32, tag="ob")
                nc.vector.tensor_tensor(ob[:], out_acc[:, tt, :], x2_t[:], ALU.add)
                nc.sync.dma_start(out[tt * P:(tt + 1) * P, :], ob[:])

        xn2_es.close()
        es.close()

    nc.compile()
    return nc


def kernel(**inputs):
    inputs = {k: np.ascontiguousarray(np.asarray(v), dtype=np.float32)
              for k, v in inputs.items()}
    ln_affine = not (
        np.all(inputs["ln1_g"] == 1.0) and np.all(inputs["ln1_b"] == 0.0)
        and np.all(inputs["ln2_g"] == 1.0) and np.all(inputs["ln2_b"] == 0.0))
    key = ("nc", ln_affine, MLP_BF16)
    if key not in _CACHE:
        _CACHE[key] = _build(ln_affine=ln_affine, mlp_bf16=MLP_BF16)
    nc = _CACHE[key]

    x = inputs["x"]
    def tile_w(W, n_out, m):
        # [Din, Dout] -> [Dout/m, 128, Din/128, m]
        Din, Dout = W.shape
        return np.ascontiguousarray(
            W.reshape(Din // P, P, n_out, m).transpose(2, 1, 0, 3))

    shared = {
        "Wq": tile_w(inputs["Wq"], KD, P), "Wk": tile_w(inputs["Wk"], KD, P),
        "Wv": tile_w(inputs["Wv"], NG, 256), "Wo": tile_w(inputs["Wo"], KD, P),
        "W1": tile_w(inputs["W1"], FT, P),
        "W2": (tile_w(inputs["W2"], KD, P).astype(__import__("ml_dtypes").bfloat16)
               if MLP_BF16 else tile_w(inputs["W2"], KD, P)),
        "bq": inputs["bq"], "bk": inputs["bk"], "bv": inputs["bv"], "bo": inputs["bo"],
        "b1": inputs["b1"], "b2": inputs["b2"],
        "g1": inputs["ln1_g"], "be1": inputs["ln1_b"],
        "g2": inputs["ln2_g"], "be2": inputs["ln2_b"],
    }
    in_maps = []
    for c in range(8):
        b, half = c // 2, c % 2
        m = dict(shared)
        # query half first; attention is permutation-invariant over kv order
        m["xkv"] = np.ascontiguousarray(
            np.concatenate([x[b, half * NQ:(half + 1) * NQ, :],
                            x[b, (1 - half) * NQ:(2 - half) * NQ, :]], axis=0))
        in_maps.append(m)

    trace = bool(int(os.environ.get("KERNEL_TRACE", "0")))
    kw = {}
    if trace:
        kw = dict(trace=True, tmpdir=os.environ.get("KERNEL_TRACE_DIR") or None)
    res = bass_utils.run_bass_kernel_spmd(nc, in_maps, core_ids=list(range(8)), **kw)
    _CACHE["last_results"] = res
    _CACHE["nc"] = nc
    _CACHE["last_in_maps"] = in_maps

    outa = np.empty((B, S, D), dtype=np.float32)
    for c in range(8):
        b, half = c // 2, c % 2
        outa[b, half * NQ:(half + 1) * NQ, :] = res.results[c]["out"]
    return outa


# revision 32
# speedup vs baseline: 1.0167x; 1.0048x over previous
"""Trainium2 Bass kernel for a dense transformer encoder block (B=4, S=2048,
D=1024, H=16, MLP=4096).

Sharding: 8 cores = 4 batch elements x 2 query-halves, no collectives. Each
core's kv sequence is host-reordered so its 1024 query tokens come first
(attention is permutation-invariant over keys), so Q/residual tensors are
plain slices of the kv set. K/V are computed for the full 2048-token sequence
(~6% duplicated FLOPs vs. perfect sharding).

Per-core dataflow is feature-major ("T" = [feature, token]) so every matmul
has contraction dim 128 on partitions (sub-128-contraction matmuls fail to
load on this stack, all dtypes):
  LN1 (token-major, bn_stats) -> PE-transpose -> xnT            [phase 1]
  per head-group of 4 heads: Q/K/V projections from xnT         [phase 3]
    scores^T = KT_pair^T @ Qpad   (zero-padded rhs selects one head
                                   of the packed pair; K=128 kept)
    exp on ACT, scale=1/8 fused, both heads in one [128,1024] op -> f32r
    AV+den fused: lhsT = [V_head | 1 | 0] so psum rows 0:64 = V^T e and
      row 64 = sum(e); one augmented matmul per (ktile, head)
    reciprocal of row 64, partition-broadcast via DRAM round-trip DMA
      (stride-0 partition APs are DRAM-only), multiply -> RT; head B's
      rows shift 0:64 -> 64:128 via a small SBUF->SBUF DMA
  O-proj +bo, PE-transpose back, +residual -> x2 -> DRAM        [phase 4a]
  LN2 on x2 -> PE-transpose -> xn2T                             [phase 4b]
  MLP: h1 (+b1 and exact-erf Gelu fused on ACT), h2 (+b2),      [phase 5]
    PE-transpose back, +x2 residual -> out

Numerics: matmuls in float32r (TF32-class, ~1.5e-4 rel err, full PE rate at
free-dim >= 256; requires producers typed f32r), fp32 PSUM accumulation,
fp32 layernorm/softmax scalars. End-to-end rel err ~1.4e-4.

Weights are host-retiled to [tile, partition, kd, m] so each weight-tile DMA
is one contiguous block (4KB per-partition chunks). LN affine (g=1, b=0 for
this problem's inputs) is skipped at build time when the host detects
identity values; a full-affine variant is built otherwise.

Cost-model (TimelineSim) span: ~866 us/core; PE busy ~820 us (the binding
engine; attention runs at 50% array utilization, the price of the K=128
constraint with DH=64 heads and no working sub-128 row/col tiling).
"""

import os
import sys

sys.path.insert(0, "/opt/trn_rl_repo")

from contextlib import ExitStack

import numpy as np

import concourse.bass as bass
import concourse.tile as tile
from concourse import bacc, bass_utils, mybir
from concourse.masks import make_identity

F32 = mybir.dt.float32
F32R = mybir.dt.float32r
BF16 = mybir.dt.bfloat16
AF = mybir.ActivationFunctionType
ALU = mybir.AluOpType

B, S, D = 4, 2048, 1024
H, DH, MLP = 16, 64, 4096
P = 128
KD = D // P            # 8 partition tiles over D
FT = MLP // P          # 32 partition tiles over MLP dim
NQ = S // 2            # 1024 query tokens per core
ST = S // P            # 16 kv token tiles
QTT = NQ // P          # 8 q token tiles
QS = 512               # free-dim slice
NQS = NQ // QS         # 2
NKS = S // QS          # 4
NG = 4                 # head groups
EPS = 1e-6
DEBUG = bool(int(os.environ.get("KERNEL_DEBUG", "0")))
MLP_BF16 = bool(int(os.environ.get("KERNEL_MLP_BF16", "0")))

_CACHE = {}


def _build(ln_affine=True, mlp_bf16=True):
    nc = bacc.Bacc(None, target_bir_lowering=False, debug=False, num_devices=8)

    xkv = nc.dram_tensor("xkv", [S, D], F32, kind="ExternalInput").ap()
    # weights arrive host-tiled: [tile, p, kd, m] so each SBUF weight tile is
    # one contiguous DRAM block (4KB+ per-partition DMA chunks)
    Wq = nc.dram_tensor("Wq", [KD, P, KD, P], F32R, kind="ExternalInput").ap()
    Wk = nc.dram_tensor("Wk", [KD, P, KD, P], F32R, kind="ExternalInput").ap()
    Wv = nc.dram_tensor("Wv", [NG, P, KD, 256], F32R, kind="ExternalInput").ap()
    Wo = nc.dram_tensor("Wo", [KD, P, KD, P], F32R, kind="ExternalInput").ap()
    W1 = nc.dram_tensor("W1", [FT, P, KD, P], F32R, kind="ExternalInput").ap()
    W2 = nc.dram_tensor("W2", [KD, P, FT, P], BF16 if mlp_bf16 else F32R, kind="ExternalInput").ap()
    bq = nc.dram_tensor("bq", [D], F32, kind="ExternalInput").ap()
    bk = nc.dram_tensor("bk", [D], F32, kind="ExternalInput").ap()
    bv = nc.dram_tensor("bv", [D], F32, kind="ExternalInput").ap()
    bo = nc.dram_tensor("bo", [D], F32, kind="ExternalInput").ap()
    b1 = nc.dram_tensor("b1", [MLP], F32, kind="ExternalInput").ap()
    b2 = nc.dram_tensor("b2", [D], F32, kind="ExternalInput").ap()
    g1 = nc.dram_tensor("g1", [D], F32, kind="ExternalInput").ap()
    be1 = nc.dram_tensor("be1", [D], F32, kind="ExternalInput").ap()
    g2 = nc.dram_tensor("g2", [D], F32, kind="ExternalInput").ap()
    be2 = nc.dram_tensor("be2", [D], F32, kind="ExternalInput").ap()
    out = nc.dram_tensor("out", [NQ, D], F32, kind="ExternalOutput").ap()

    dbg = {}
    if DEBUG:
        dbg["xnkvT"] = nc.dram_tensor("d_xnkvT", [P, KD, S], F32R, kind="ExternalOutput").ap()
        dbg["qt0"] = nc.dram_tensor("d_qt0", [P, 2, NQ], F32R, kind="ExternalOutput").ap()
        dbg["kt0"] = nc.dram_tensor("d_kt0", [P, 2, S], F32R, kind="ExternalOutput").ap()
        dbg["v0"] = nc.dram_tensor("d_v0", [P, ST, 2, 2, P], F32R, kind="ExternalOutput").ap()
        dbg["rt"] = nc.dram_tensor("d_rt", [P, KD, NQ], F32R, kind="ExternalOutput").ap()
        dbg["e0"] = nc.dram_tensor("d_e0", [P, QS], F32R, kind="ExternalOutput").ap()
        dbg["s0"] = nc.dram_tensor("d_s0", [P, QS], F32, kind="ExternalOutput").ap()
        dbg["av0"] = nc.dram_tensor("d_av0", [65, 2, QS], F32, kind="ExternalOutput").ap()
        dbg["x2"] = nc.dram_tensor("d_x2", [P, QTT, D], F32, kind="ExternalOutput").ap()

    def bcast_ap(vec):
        # [D] dram vector -> [128, D] partition-replicated DMA source
        return bass.AP(tensor=vec.tensor, offset=vec.offset, ap=[[0, P]] + list(vec.ap))



    with tile.TileContext(nc) as tc:
        es = ExitStack()
        params = es.enter_context(tc.tile_pool(name="params", bufs=1))
        dramp = es.enter_context(tc.tile_pool(name="dram", bufs=1, space="DRAM"))
        x2d = dramp.tile([P, QTT, D], F32)

        ident_f = params.tile([P, P], F32)
        make_identity(nc, ident_f)
        ident = params.tile([P, P], F32R)
        nc.vector.tensor_copy(ident[:], ident_f[:])
        ones_f = params.tile([P, 1], F32)
        nc.vector.memset(ones_f[:, 0:1], 1.0)

        def pvec(v, n, nm):  # [n*128] -> [128, n] (dim o*128+p -> [p, o])
            t = params.tile([P, n], F32, name=nm)
            nc.sync.dma_start(t[:], v.rearrange("(o p) -> p o", p=P))
            return t

        bq_t = pvec(bq, KD, "bq_t")
        bk_t = pvec(bk, KD, "bk_t")
        bo_t = pvec(bo, KD, "bo_t")
        b2_t = pvec(b2, KD, "b2_t")
        b1_t = pvec(b1, FT, "b1_t")
        bv_rep = params.tile([P, D], F32)
        nc.gpsimd.dma_start(bv_rep[:], bcast_ap(bv))

        rt_es = ExitStack()
        rtp = rt_es.enter_context(tc.tile_pool(name="rt", bufs=1))
        RT = rtp.tile([P, KD, NQ], F32R)

        xn_es = ExitStack()
        xnp = xn_es.enter_context(tc.tile_pool(name="xn", bufs=1))
        xn_kvT = xnp.tile([P, KD, S], F32R)

        # ---- Phase 1: LN1 + transpose to feature-major ----
        with tc.tile_pool(name="p1tmp", bufs=4) as p1t, \
             tc.tile_pool(name="p1s", bufs=4) as p1s, \
             tc.tile_pool(name="ln1", bufs=1) as ln1p, \
             tc.tile_pool(name="p1ps", bufs=6, space="PSUM") as ps1:
            g1_rep = ln1p.tile([P, D], F32)
            nc.gpsimd.dma_start(g1_rep[:], bcast_ap(g1))
            be1_rep = ln1p.tile([P, D], F32)
            nc.gpsimd.dma_start(be1_rep[:], bcast_ap(be1))
            eps_t = ln1p.tile([P, 1], F32)
            nc.vector.memset(eps_t[:], EPS)

            for t in range(ST):
                x_t = p1t.tile([P, D], F32, tag="x_t")
                nc.sync.dma_start(x_t[:], xkv[t * P:(t + 1) * P, :])
                stats = p1s.tile([P, 2, 6], F32, tag="stats")
                xv = x_t[:].rearrange("p (s f) -> p s f", s=2)
                for s in range(2):
                    nc.vector.bn_stats(stats[:, s, :], xv[:, s, :])
                mv = p1s.tile([P, 2], F32, tag="mv")
                nc.vector.bn_aggr(mv[:], stats[:])
                std = p1s.tile([P, 1], F32, tag="std")
                nc.scalar.activation(std[:], mv[:, 1:2], AF.Sqrt, bias=eps_t[:])
                nc.vector.reciprocal(std[:], std[:])
                xn_t = p1t.tile([P, D], F32R, tag="xn_t")
                nc.vector.tensor_scalar(
                    xn_t[:], x_t[:], scalar1=mv[:, 0:1], scalar2=std[:],
                    op0=ALU.subtract, op1=ALU.mult)
                if ln_affine:
                    nc.vector.tensor_tensor(xn_t[:], xn_t[:], g1_rep[:], ALU.mult)
                    nc.vector.tensor_tensor(xn_t[:], xn_t[:], be1_rep[:], ALU.add)
                for j2 in range(KD // 2):
                    pst = ps1.tile([P, 2, P], F32, tag="tp")
                    for h in range(2):
                        nc.tensor.transpose(
                            pst[:, h, :].bitcast(F32R),
                            xn_t[:, (2 * j2 + h) * P:(2 * j2 + h + 1) * P], ident[:])
                    nc.vector.tensor_copy(
                        xn_kvT[:, 2 * j2:2 * j2 + 2, t * P:(t + 1) * P], pst[:])

        if DEBUG:
            nc.sync.dma_start(dbg["xnkvT"], xn_kvT[:])

        # ---- Phase 3: per-group QKV projection + attention ----
        with tc.tile_pool(name="kv", bufs=1) as kvp, \
             tc.tile_pool(name="wst", bufs=2) as wsp, \
             tc.tile_pool(name="expp", bufs=2) as expp, \
             tc.tile_pool(name="qpad", bufs=1) as qpp, \
             tc.tile_pool(name="rcbc", bufs=1) as rcp, \
             tc.tile_pool(name="aps", bufs=1, space="PSUM") as aps:

            zsc = qpp.tile([P, QS], F32)
            nc.vector.memset(zsc[:], 0.0)
            qpadA = [qpp.tile([P, QS], F32R, name=f"qpadA{i}") for i in range(1)]
            qpadB = [qpp.tile([P, QS], F32R, name=f"qpadB{i}") for i in range(1)]
            for i in range(1):
                nc.vector.tensor_copy(qpadA[i][:], zsc[:])
                nc.vector.tensor_copy(qpadB[i][:], zsc[:])

            QT_g = kvp.tile([P, 2, NQ], F32R)
            KT_g = kvp.tile([P, 2, S], F32R)
            # per (toktile, pair, head j): [V_head(64) | 1 | 0(63)]
            V_gp = kvp.tile([P, ST, 2, 2, P], F32R)
            for t in range(ST):
                nc.vector.tensor_copy(
                    V_gp[:, t], zsc[:].rearrange("p (a b m) -> p a b m", a=2, b=2))
            one_r = qpp.tile([P, 1], F32R)
            nc.vector.tensor_copy(one_r[:], ones_f[:, 0:1])
            for t in range(ST):
                for pi in range(2):
                    for j in range(2):
                        nc.vector.tensor_copy(V_gp[:, t, pi, j, 64:65], one_r[:])
            it_count = 0

            for g in range(NG):
                for pl in range(2):   # head pairs 2g, 2g+1
                    pr = 2 * g + pl
                    wq_t = wsp.tile([P, KD, P], F32R, tag="wq_t")
                    nc.sync.dma_start(wq_t[:], Wq[pr])
                    for q in range(NQS):
                        ps = aps.tile([P, QS], F32, tag="pp", bufs=2)
                        for kd in range(KD):
                            nc.tensor.matmul(
                                ps[:], wq_t[:, kd, :], xn_kvT[:, kd, q * QS:(q + 1) * QS],
                                start=(kd == 0), stop=(kd == KD - 1))
                        nc.vector.tensor_scalar_add(
                            QT_g[:, pl, q * QS:(q + 1) * QS], ps[:], bq_t[:, pr:pr + 1])
                    wk_t = wsp.tile([P, KD, P], F32R, tag="wk_t")
                    nc.sync.dma_start(wk_t[:], Wk[pr])
                    for q in range(NKS):
                        ps = aps.tile([P, QS], F32, tag="pp", bufs=2)
                        for kd in range(KD):
                            nc.tensor.matmul(
                                ps[:], wk_t[:, kd, :], xn_kvT[:, kd, q * QS:(q + 1) * QS],
                                start=(kd == 0), stop=(kd == KD - 1))
                        nc.vector.tensor_scalar_add(
                            KT_g[:, pl, q * QS:(q + 1) * QS], ps[:], bk_t[:, pr:pr + 1])
                wv_t = wsp.tile([P, KD, 256], F32R, tag="wv_t", bufs=1)
                nc.sync.dma_start(wv_t[:], Wv[g])
                for t in range(ST):
                    ps = aps.tile([P, QS], F32, tag="pp", bufs=2)
                    for kd in range(KD):
                        nc.tensor.matmul(
                            ps[:, 0:256], xn_kvT[:, kd, t * P:(t + 1) * P], wv_t[:, kd, :],
                            start=(kd == 0), stop=(kd == KD - 1))
                    for pi in range(2):
                        nc.vector.tensor_tensor(
                            V_gp[:, t, pi, :, 0:64],
                            ps[:, pi * 128:(pi + 1) * 128].rearrange("p (j m) -> p j m", j=2),
                            bv_rep[:, g * 256 + pi * 128:g * 256 + (pi + 1) * 128].rearrange(
                                "p (j m) -> p j m", j=2), ALU.add)

                if DEBUG and g == 0:
                    nc.sync.dma_start(dbg["kt0"], KT_g[:])
                    nc.sync.dma_start(dbg["v0"], V_gp[:])
                    nc.sync.dma_start(dbg["qt0"], QT_g[:])

                for q in range(NQS):
                    for pl in range(2):
                        pr = 2 * g + pl
                        i = it_count % 1
                        it_count += 1
                        qsl = slice(q * QS, (q + 1) * QS)
                        nc.vector.tensor_copy(qpadA[i][0:64, :], QT_g[0:64, pl, qsl])
                        nc.vector.tensor_copy(qpadB[i][64:128, :], QT_g[64:128, pl, qsl])
                        av1 = aps.tile([P, QS], F32, tag="av1")
                        av2 = aps.tile([P, QS], F32, tag="av2")
                        for kt in range(ST):
                            ktsl = slice(kt * P, (kt + 1) * P)
                            sAB = aps.tile([P, 2, QS], F32, tag="sAB", bufs=2)
                            nc.tensor.matmul(sAB[:, 0, :], KT_g[:, pl, ktsl], qpadA[i][:],
                                             start=True, stop=True)
                            nc.tensor.matmul(sAB[:, 1, :], KT_g[:, pl, ktsl], qpadB[i][:],
                                             start=True, stop=True)
                            eAB = expp.tile([P, 2, QS], F32R, tag="eAB")
                            nc.scalar.activation(eAB[:], sAB[:], AF.Exp, scale=0.125)
                            eA = eAB[:, 0, :]
                            eB = eAB[:, 1, :]
                            if DEBUG and g == 0 and q == 0 and pl == 0 and kt == 0:
                                nc.sync.dma_start(dbg["e0"], eA)
                                s0c = rcp.tile([P, QS], F32, tag="s0c")
                                nc.vector.tensor_copy(s0c[:], sAB[:, 0, :])
                                nc.sync.dma_start(dbg["s0"], s0c[:])
                            st, sp = (kt == 0), (kt == ST - 1)
                            nc.tensor.matmul(av1[:], V_gp[:, kt, pl, 0, :], eA,
                                             start=st, stop=sp, skip_group_check=True)
                            nc.tensor.matmul(av2[:], V_gp[:, kt, pl, 1, :], eB,
                                             start=st, stop=sp, skip_group_check=True)
                        # free the av psums fast: copy to SBUF, divide from there
                        avc = rcp.tile([65, 2, QS], F32, tag="avc")
                        nc.vector.tensor_copy(avc[0:65, 0, :], av1[0:65, :])
                        nc.vector.tensor_copy(avc[0:65, 1, :], av2[0:65, :])
                        nc.vector.reciprocal(avc[64:65, 0, :], avc[64:65, 0, :])
                        nc.vector.reciprocal(avc[64:65, 1, :], avc[64:65, 1, :])
                        rcd = dramp.tile([2, QS], F32, tag="rcd", bufs=2)
                        nc.sync.dma_start(rcd[0:1, :], avc[64:65, 0, :])
                        nc.sync.dma_start(rcd[1:2, :], avc[64:65, 1, :])
                        bcA = rcp.tile([64, QS], F32, tag="bcA")
                        bcB = rcp.tile([64, QS], F32, tag="bcB")

                        def _b64(row_ap):
                            return bass.AP(tensor=row_ap.tensor, offset=row_ap.offset,
                                           ap=[[0, 64]] + list(row_ap.ap)[1:])

                        nc.sync.dma_start(bcA[:], _b64(rcd[0:1, :]))
                        nc.sync.dma_start(bcB[:], _b64(rcd[1:2, :]))
                        if DEBUG and g == 0 and q == 0 and pl == 0:
                            nc.sync.dma_start(dbg["av0"], avc[:])
                        nc.vector.tensor_tensor(RT[0:64, pr, qsl], avc[0:64, 0, :], bcA[:], ALU.mult)
                        stB = rcp.tile([64, QS], F32R, tag="stB")
                        nc.vector.tensor_tensor(stB[:], avc[0:64, 1, :], bcB[:], ALU.mult)
                        nc.sync.dma_start(RT[64:128, pr, qsl], stB[:])

        xn_es.close()

        if DEBUG:
            nc.sync.dma_start(dbg["rt"], RT[:])

        # ---- Phase 4a: O-projection + residual -> x2 (DRAM) ----
        with tc.tile_pool(name="p4tmp", bufs=2) as p4t, \
             tc.tile_pool(name="p4ps", bufs=2, space="PSUM") as ps4, \
             tc.tile_pool(name="p4tps", bufs=6, space="PSUM") as ps4t:
            for q in range(NQS):
                attnT = p4t.tile([P, KD, QS], F32R, tag="attnT")
                for mt in range(KD):
                    wo_t = p4t.tile([P, KD, P], F32R, tag="wo_t")
                    nc.sync.dma_start(wo_t[:], Wo[mt])
                    ps = ps4.tile([P, QS], F32, tag="pp")
                    for kd in range(KD):
                        nc.tensor.matmul(
                            ps[:], wo_t[:, kd, :], RT[:, kd, q * QS:(q + 1) * QS],
                            start=(kd == 0), stop=(kd == KD - 1))
                    nc.vector.tensor_scalar_add(
                        attnT[:, mt, :], ps[:], bo_t[:, mt:mt + 1])
                for j in range(QS // P):
                    tt = q * (QS // P) + j
                    xr_t = p4t.tile([P, D], F32, tag="xr_t")
                    nc.sync.dma_start(xr_t[:], xkv[tt * P:(tt + 1) * P, :])
                    x2_t = p4t.tile([P, D], F32, tag="x2_t")
                    for m2 in range(KD // 2):
                        pst = ps4t.tile([P, 2, P], F32, tag="tp")
                        for h in range(2):
                            nc.tensor.transpose(
                                pst[:, h, :].bitcast(F32R),
                                attnT[:, 2 * m2 + h, j * P:(j + 1) * P], ident[:])
                        nc.vector.tensor_tensor(
                            x2_t[:, 2 * m2 * P:(2 * m2 + 2) * P],
                            pst[:].rearrange("p a m -> p (a m)"),
                            xr_t[:, 2 * m2 * P:(2 * m2 + 2) * P], ALU.add)
                    nc.sync.dma_start(x2d[:, tt, :], x2_t[:])
                    if DEBUG:
                        nc.sync.dma_start(dbg["x2"][:, tt, :], x2_t[:])
        rt_es.close()

        # ---- Phase 4b: LN2 -> xn2T ----
        xn2_es = ExitStack()
        xn2p = xn2_es.enter_context(tc.tile_pool(name="xn2", bufs=1))
        xn2T_h = [xn2p.tile([P, KD, QS], F32R, name=f"xn2T{h}") for h in range(NQS)]
        with tc.tile_pool(name="p4btmp", bufs=4) as p4bt, \
             tc.tile_pool(name="p4bs", bufs=4) as p4bs, \
             tc.tile_pool(name="ln2", bufs=1) as ln2p, \
             tc.tile_pool(name="p4bps", bufs=6, space="PSUM") as ps4b:
            g2_rep = ln2p.tile([P, D], F32)
            nc.gpsimd.dma_start(g2_rep[:], bcast_ap(g2))
            be2_rep = ln2p.tile([P, D], F32)
            nc.gpsimd.dma_start(be2_rep[:], bcast_ap(be2))
            eps2_t = ln2p.tile([P, 1], F32)
            nc.vector.memset(eps2_t[:], EPS)

            for tt in range(QTT):
                x2_t = p4bt.tile([P, D], F32, tag="x2_t")
                nc.sync.dma_start(x2_t[:], x2d[:, tt, :])
                stats = p4bs.tile([P, 2, 6], F32, tag="stats2")
                xv = x2_t[:].rearrange("p (s f) -> p s f", s=2)
                for s in range(2):
                    nc.vector.bn_stats(stats[:, s, :], xv[:, s, :])
                mv = p4bs.tile([P, 2], F32, tag="mv2")
                nc.vector.bn_aggr(mv[:], stats[:])
                std = p4bs.tile([P, 1], F32, tag="std2")
                nc.scalar.activation(std[:], mv[:, 1:2], AF.Sqrt, bias=eps2_t[:])
                nc.vector.reciprocal(std[:], std[:])
                xn2_t = p4bt.tile([P, D], F32R, tag="xn2_t")
                nc.vector.tensor_scalar(
                    xn2_t[:], x2_t[:], scalar1=mv[:, 0:1], scalar2=std[:],
                    op0=ALU.subtract, op1=ALU.mult)
                if ln_affine:
                    nc.vector.tensor_tensor(xn2_t[:], xn2_t[:], g2_rep[:], ALU.mult)
                    nc.vector.tensor_tensor(xn2_t[:], xn2_t[:], be2_rep[:], ALU.add)
                hs_i, loc = tt // (QS // P), (tt % (QS // P)) * P
                for j2 in range(KD // 2):
                    pst = ps4b.tile([P, 2, P], F32, tag="tp")
                    for h in range(2):
                        nc.tensor.transpose(
                            pst[:, h, :].bitcast(F32R),
                            xn2_t[:, (2 * j2 + h) * P:(2 * j2 + h + 1) * P], ident[:])
                    nc.vector.tensor_copy(
                        xn2T_h[hs_i][:, 2 * j2:2 * j2 + 2, loc:loc + P], pst[:])

        # ---- Phase 5: MLP (h1 in bf16, single full-width token pass) ----
        with tc.tile_pool(name="p5tmp", bufs=3) as p5t, \
             tc.tile_pool(name="h1", bufs=1) as h1p, \
             tc.tile_pool(name="w2st", bufs=2) as w2p, \
             tc.tile_pool(name="p5ps", bufs=2, space="PSUM") as ps5, \
             tc.tile_pool(name="p5tps", bufs=4, space="PSUM") as ps5t:
            mdt = BF16 if mlp_bf16 else F32R
            n_hslice = 1 if mlp_bf16 else NQS
            HW_ = NQ // n_hslice
            out_acc = h1p.tile([P, QTT, D], F32)
            for hs in range(n_hslice):
                h1T = h1p.tile([P, FT, HW_], mdt, tag="h1T")
                for ft in range(FT):
                    w1_t = p5t.tile([P, KD, P], F32R, tag="w1_t")
                    nc.sync.dma_start(w1_t[:], W1[ft])
                    for sl in range(HW_ // QS):
                        gsl = (hs * HW_ + sl * QS) // QS
                        ps = ps5.tile([P, QS], F32, tag="pp")
                        for kd in range(KD):
                            nc.tensor.matmul(
                                ps[:], w1_t[:, kd, :], xn2T_h[gsl][:, kd, :],
                                start=(kd == 0), stop=(kd == KD - 1))
                        nc.scalar.activation(h1T[:, ft, sl * QS:(sl + 1) * QS], ps[:],
                                             AF.Gelu, bias=b1_t[:, ft:ft + 1])
                for mt in range(KD):
                    w2_t = w2p.tile([P, FT, P], mdt, tag="w2_t")
                    nc.sync.dma_start(w2_t[:], W2[mt])
                    for sl in range(HW_ // QS):
                        ssl_loc = slice(sl * QS, (sl + 1) * QS)
                        ps = ps5.tile([P, QS], F32, tag="pp")
                        for ft in range(FT):
                            nc.tensor.matmul(
                                ps[:], w2_t[:, ft, :], h1T[:, ft, ssl_loc],
                                start=(ft == 0), stop=(ft == FT - 1))
                        outT = p5t.tile([P, QS], F32R, tag="outT", bufs=2)
                        nc.vector.tensor_scalar_add(outT[:], ps[:], b2_t[:, mt:mt + 1])
                        for j in range(QS // P):
                            tt = hs * (HW_ // P) + sl * (QS // P) + j
                            pst = ps5t.tile([P, P], F32, tag="tp")
                            nc.tensor.transpose(pst[:].bitcast(F32R),
                                                outT[:, j * P:(j + 1) * P], ident[:])
                            nc.vector.tensor_copy(out_acc[:, tt, mt * P:(mt + 1) * P], pst[:])
            for tt in range(QTT):
                x2_t = p5t.tile([P, D], F32, tag="x2r_t")
                nc.sync.dma_start(x2_t[:], x2d[:, tt, :])
                ob = p5t.tile([P, D], F32, tag="ob")
                nc.vector.tensor_tensor(ob[:], out_acc[:, tt, :], x2_t[:], ALU.add)
                nc.sync.dma_start(out[tt * P:(tt + 1) * P, :], ob[:])

        xn2_es.close()
        es.close()

    nc.compile()
    return nc


def kernel(**inputs):
    inputs = {k: np.ascontiguousarray(np.asarray(v), dtype=np.float32)
              for k, v in inputs.items()}
    ln_affine = not (
        np.all(inputs["ln1_g"] == 1.0) and np.all(inputs["ln1_b"] == 0.0)
        and np.all(inputs["ln2_g"] == 1.0) and np.all(inputs["ln2_b"] == 0.0))
    key = ("nc", ln_affine, MLP_BF16)
    if key not in _CACHE:
        _CACHE[key] = _build(ln_affine=ln_affine, mlp_bf16=MLP_BF16)
    nc = _CACHE[key]

    x = inputs["x"]
    def tile_w(W, n_out, m):
        # [Din, Dout] -> [Dout/m, 128, Din/128, m]
        Din, Dout = W.shape
        return np.ascontiguousarray(
            W.reshape(Din // P, P, n_out, m).transpose(2, 1, 0, 3))

    shared = {
        "Wq": tile_w(inputs["Wq"], KD, P), "Wk": tile_w(inputs["Wk"], KD, P),
        "Wv": tile_w(inputs["Wv"], NG, 256), "Wo": tile_w(inputs["Wo"], KD, P),
        "W1": tile_w(inputs["W1"], FT, P),
        "W2": (tile_w(inputs["W2"], KD, P).astype(__import__("ml_dtypes").bfloat16)
               if MLP_BF16 else tile_w(inputs["W2"], KD, P)),
        "bq": inputs["bq"], "bk": inputs["bk"], "bv": inputs["bv"], "bo": inputs["bo"],
        "b1": inputs["b1"], "b2": inputs["b2"],
        "g1": inputs["ln1_g"], "be1": inputs["ln1_b"],
        "g2": inputs["ln2_g"], "be2": inputs["ln2_b"],
    }
    in_maps = []
    for c in range(8):
        b, half = c // 2, c % 2
        m = dict(shared)
        # query half first; attention is permutation-invariant over kv order
        m["xkv"] = np.ascontiguousarray(
            np.concatenate([x[b, half * NQ:(half + 1) * NQ, :],
                            x[b, (1 - half) * NQ:(2 - half) * NQ, :]], axis=0))
        in_maps.append(m)

    trace = bool(int(os.environ.get("KERNEL_TRACE", "0")))
    kw = {}
    if trace:
        kw = dict(trace=True, tmpdir=os.environ.get("KERNEL_TRACE_DIR") or None)
    res = bass_utils.run_bass_kernel_spmd(nc, in_maps, core_ids=list(range(8)), **kw)
    _CACHE["last_results"] = res
    _CACHE["nc"] = nc
    _CACHE["last_in_maps"] = in_maps

    outa = np.empty((B, S, D), dtype=np.float32)
    for c in range(8):
        b, half = c // 2, c % 2
        outa[b, half * NQ:(half + 1) * NQ, :] = res.results[c]["out"]
    return outa
